# revision 7
# baseline (speedup 1.0000x reference)
"""Trainium2 Bass kernel for BasicMotionEncoder (RAFT motion encoder).

Network (all stride-1 convs, NCHW, fp32 in/out):
    cor  = relu(conv1x1(corr, wc1, bc1))          # [B,256,H,W]
    cor  = relu(conv3x3(cor,  wc2, bc2, pad 1))   # [B,192,H,W]
    flo  = relu(conv7x7(flow, wf1, bf1, pad 3))   # [B,128,H,W]
    flo  = relu(conv3x3(flo,  wf2, bf2, pad 1))   # [B,64,H,W]
    out  = relu(conv3x3(cat(cor,flo), wo, bo, 1)) # [B,126,H,W]
    return cat(out, flow)                         # [B,128,H,W]

Sharding: pure data parallel, one image per NeuronCore (B=8, 8 cores).
Each core streams its image through the 5-layer pipeline in 24 4-row
blocks with all intermediate activations resident in SBUF at full image
height (no pass/halo recompute); only corr is streamed in and the
126-channel output written back.  Convs are PE matmuls with channels on
the partition dim: for each tap the shifted input window is a strided AP
into a zero-padded SBUF image, accumulated in PSUM over taps and k-tiles
(bf16 operands, fp32 PSUM — bf16 is 1 row/cycle through the PE like
fp32r but halves LDWEIGHTS + operand SBUF traffic; rel-err ~4e-3 vs the
2e-2 budget).  All M<=64 weights are padded to M=128: half-width
(col_grp=h0) LDWEIGHTS mixed with full-width ones break weight-load/
matmul overlap and cost ~110ns per matmul, while matmul time depends
only on the free dim, not M.  The final concat of `flow` into channels
126:128 happens on the host.

f2 runs in fp8(e4m3) MatmulPerfMode.DoubleRow: each DR matmul contracts
TWO taps (lhsT [K,2,M], rhs [K,2,N]) in the same 512-cycle stream,
taking f2 from 9 matmuls/block to 4 paired + 1 single (4608c -> 2560c,
~21us).  End-to-end rel-err with f2-only fp8 is ~9e-3 (sim-verified)
vs the 2e-2 budget.  The PE ifmap walker only supports the pair dim
plus ONE more (merged) dim, so the paired windows must be contiguous:
f1's output is stored as THREE 128-wide column-shifted planes
flo3[plane dw][c] = flo[c+dw-1] (f1's ACT writes the center plane, the
otherwise-idle DVE copies the +-1 shifted planes), making every 3x3-tap
window a flat 512-run and the pair AP [K, (delta,2), (1,512)].
"""

import ml_dtypes
import numpy as np

import bass_rust
import concourse.mybir as mybir
import concourse.tile as tile
from concourse import bacc
from concourse.bass_utils import run_bass_kernel_spmd

H, W = 96, 128
CIN_CORR = 324
WP = W + 2  # pad-1 padded row width (3x3 convs)
NBLK = H // 4  # 4-row output blocks streamed through the pipeline
F32 = mybir.dt.float32
BF16 = mybir.dt.bfloat16
F8 = mybir.dt.float8e4
NPBF16 = ml_dtypes.bfloat16
NPF8 = ml_dtypes.float8_e4m3fn
RELU = mybir.ActivationFunctionType.Relu
COPY = mybir.ActivationFunctionType.Copy
DR = mybir.MatmulPerfMode.DoubleRow

ZELEMS = 512  # zeros tile length (>= 2 x buffer rows; 512 for warmup rhs)
# Zero-weight warmup matmuls: bridge PE start (~8.1us) to the first
# stack-chunk DMA arrival (~13-13.7us) with NO gap, so the 1.2->2.4GHz
# ramp (3us continuous) completes on useless work and the real stream
# starts at full clock.  Any PE gap resets the ramp: ~3us of 427ns
# matmuls.  18 x 512-free: ~7 at 427ns (ramping) + ~11 at 213ns.
NWARM = 18

# flo3 plane geometry: [128, 3, H+4, 128]; plane stride in elements.
FLO_ROWS = H + 4
FLO_PLANE = FLO_ROWS * 128

# f2 DoubleRow tap pairing: taps (dh,dw); pair = (base_tap, partner_tap)
# with constant AP delta = (dw'-dw)*FLO_PLANE + (dh'-dh)*128.  Windows
# for tap (dh,dw) on block cc: flo3[:, dw, cc+1+dh : cc+5+dh, :].
F2_PAIRS = [((0, 0), (0, 1)), ((1, 0), (0, 2)), ((1, 1), (1, 2)), ((2, 0), (2, 1))]
F2_SINGLE = (2, 2)

# Row maps.  cor1 buffers: buffer row r holds image row r-2 (rows 1
# and H+2 are the zero-pad rows the edge taps read; 0 and H+3 unused).
# flo3 planes use the same row map at width 128 (plane dw pre-shifted
# by dw-1 columns).  catpad buffers: buffer row r holds cat row r-1
# (rows 0 and H+1 zero).


def _zero_borders(nc, buf, zrows):
    """Zero the conv-padding bytes of a padded image buffer with vector-
    engine memsets: cols {0,1} and {128,129} of every row (cols 1/128 are
    interior and overwritten by the relu writes that follow), plus the
    vertical-padding zero rows the edge taps read.  These ride the
    otherwise-idle DVE queue: as scalar-ACT copies they serialized in
    front of the first f1 relu and stalled the f2 pipeline ~2-3us."""
    for off in (0, W):
        nc.vector.memset(buf[:, :, off : off + 2], 0.0)
    for zrow in zrows:
        nc.vector.memset(buf[:, zrow : zrow + 1, :], 0.0)


def _pair_rhs(flo3, cc, base, partner):
    """rhs AP for a DoubleRow tap pair: the base tap's contiguous
    [4x128] window with an inserted (delta, 2) pair dim."""
    (dh, dw), (dh2, dw2) = base, partner
    delta = (dw2 - dw) * FLO_PLANE + (dh2 - dh) * 128
    rhs = flo3[:, dw, cc + 1 + dh : cc + 5 + dh, :].copy()
    ap = rhs.ap.to_list()  # [(pstride,128), (128,4), (1,128)]
    rhs.ap = bass_rust.VecI64Pair([ap[0], (delta, 2), (1, 512)])
    return rhs


def build_module():
    nc = bacc.Bacc(trn_type="TRN2", target_bir_lowering=False)
    # corr is zero-padded to 384 channels on the host so the three c1
    # k-tiles are a single DMA and a uniform K=128 contraction.
    corr = nc.dram_tensor("corr", [384, H, W], BF16, kind="ExternalInput").ap()
    corr_r = corr.rearrange("(kt p) h w -> p kt h w", kt=3)
    # f1 im2col K is padded 98 -> 128: partial-K LDWEIGHTS mixed with
    # full-K ones cost ~160ns per f1 matmul (same penalty class as
    # half-width col_grp loads).
    stackh = nc.dram_tensor("stackh", [128, H + 4, 128], BF16, kind="ExternalInput").ap()
    wc1p = nc.dram_tensor("wc1p", [128, 3, 256], BF16, kind="ExternalInput").ap()
    wc2p = nc.dram_tensor("wc2p", [128, 9, 2, 256], BF16, kind="ExternalInput").ap()
    wf1p = nc.dram_tensor("wf1p", [128, 128], BF16, kind="ExternalInput").ap()
    wf2p = nc.dram_tensor("wf2p", [128, 9, 128], F8, kind="ExternalInput").ap()
    wop = nc.dram_tensor("wop", [128, 2, 9, 126], BF16, kind="ExternalInput").ap()
    biasp = nc.dram_tensor("biasp", [128, 8], F32, kind="ExternalInput").ap()
    out = nc.dram_tensor("out", [126, H, W], F32, kind="ExternalOutput").ap()

    with tile.TileContext(nc) as tc:
        with (
            tc.tile_pool(name="wpool", bufs=1) as wpool,
            tc.tile_pool(name="pspool", space="PSUM", bufs=8) as pspool,
            tc.tile_pool(name="spool", bufs=4) as spool,
            tc.tile_pool(name="opool", bufs=3) as opool,
        ):
            wc1s = wpool.tile([128, 3, 256], BF16, name="wc1s")
            wc2s = wpool.tile([128, 9, 2, 256], BF16, name="wc2s")
            wf1s = wpool.tile([128, 128], BF16, name="wf1s")
            wf2s = wpool.tile([128, 9, 128], F8, name="wf2s")
            wos = wpool.tile([128, 2, 9, 126], BF16, name="wos")
            bs = wpool.tile([128, 8], F32, name="bs")
            zsb = wpool.tile([128, ZELEMS], BF16, name="zsb")
            scr = wpool.tile([128, 1], F32, name="scr")
            # full-height intermediates, written once per row (no halo)
            stack2 = wpool.tile([128, H + 4, 128], BF16, name="stack2")
            flo3 = wpool.tile([128, 3, FLO_ROWS, 128], F8, name="flo3")
            cor1a = wpool.tile([128, H + 4, WP], BF16, name="cor1a")
            cor1b = wpool.tile([128, H + 4, WP], BF16, name="cor1b")
            catpad1 = wpool.tile([128, H + 2, WP], BF16, name="catpad1")
            catpad2 = wpool.tile([128, H + 2, WP], BF16, name="catpad2")

            # --- setup.  The zeros tile is a vector-engine memset (a DMA
            # through the Activation-queue DGE lands ~14us late and the
            # zero-border ACTs -> f1 relu -> f2 chain all wait on it).
            # Each DGE delivers its transfers serially at ~2.5us per hop,
            # so every early-needed tensor sits near the head of SOME
            # queue.  All of stackh loads upfront (sync hop1 + scalar
            # hop2 + gpsimd hops 5-6) instead of trickling in-loop
            # refills: the old [16:36] chunk rode gpsimd hop 4, landed
            # ~19.7us and stalled f1(3) -> the whole in-order PE queue
            # for 5.2us.
            nc.vector.memset(zsb, 0.0)
            nc.sync.dma_start(out=stack2[:, 0:24, :], in_=stackh[:, 0:24, :])
            # Zero-weight warmup matmuls bridge the wait for the first
            # stack DMA: the PE needs ~3us of continuous execution to
            # ramp 1.2GHz -> 2.4GHz, so by the time real work arrives
            # the clock is at full speed.
            psd = pspool.tile([128, 4, 128], F32, tag="ps", name="psdum")
            zv1 = zsb.rearrange("p (a b) -> p a b", a=1)
            for i in range(NWARM):
                nc.tensor.matmul(
                    psd, zsb[:, 0:128], zv1, start=(i == 0), stop=(i == NWARM - 1)
                )
            nc.scalar.dma_start(out=wf1s, in_=wf1p)
            nc.gpsimd.dma_start(out=bs, in_=biasp)
            nc.gpsimd.dma_start(out=wf2s, in_=wf2p)
            nc.gpsimd.dma_start(out=wc1s, in_=wc1p)
            nc.scalar.dma_start(out=stack2[:, 24:52, :], in_=stackh[:, 24:52, :])
            # wc2s in two k-halves: c2(0) only needs kt=0 for its first
            # 18 matmuls, so the first half landing a hop earlier removes
            # the c2(0) stall
            nc.gpsimd.dma_start(out=wc2s[:, :, 0, :], in_=wc2p[:, :, 0, :])
            # prewarm the Relu activation table off the critical path
            nc.scalar.activation(scr, zsb[:, 0:1], RELU)
            # flo3 zero regions: vertical pad rows 1 and H+2 (all three
            # planes), plane0 col 0 (left pad), plane2 col 127 (right
            # pad).  Interior rows/cols are written by ACT/DVE each block.
            for zrow in (1, H + 2):
                nc.vector.memset(flo3[:, :, zrow : zrow + 1, :], 0.0)
            nc.vector.memset(flo3[:, 0, :, 0:1], 0.0)
            nc.vector.memset(flo3[:, 2, :, 127:128], 0.0)
            nc.scalar.dma_start(out=wc2s[:, :, 1, :], in_=wc2p[:, :, 1, :])
            nc.scalar.dma_start(out=wos, in_=wop)
            nc.gpsimd.dma_start(out=stack2[:, 52:76, :], in_=stackh[:, 52:76, :])
            nc.gpsimd.dma_start(out=stack2[:, 76:100, :], in_=stackh[:, 76:100, :])
            _zero_borders(nc, cor1a, (1, H + 2))
            _zero_borders(nc, cor1b, (1, H + 2))
            _zero_borders(nc, catpad1, (0, H + 1))
            _zero_borders(nc, catpad2, (0, H + 1))

            def emit_f1(rr):
                # 7x7 conv, 2 -> 128 channels.  Input is a full host-side
                # im2col: stack2 partition cin*49+dh*7+dw holds the
                # zero-padded flow image shifted by (dh, dw), so one K=98
                # matmul computes a whole block.  The relu writes the fp8
                # center plane of flo3; the DVE then copies the +-1
                # column-shifted planes the f2 DoubleRow windows read.
                ps = pspool.tile([128, 4, 128], F32, tag="ps", name=f"psf1_{rr}")
                i = rr + 2
                nc.tensor.matmul(ps, wf1s, stack2[:, i : i + 4, :], start=True, stop=True)
                nc.scalar.activation(flo3[:, 1, i : i + 4, :], ps, RELU, bias=bs[:, 4:5])
                nc.vector.tensor_copy(
                    flo3[:, 0, i : i + 4, 1:128], flo3[:, 1, i : i + 4, 0:127]
                )
                nc.vector.tensor_copy(
                    flo3[:, 2, i : i + 4, 0:127], flo3[:, 1, i : i + 4, 1:128]
                )

            def emit_f2(cc):
                # 3x3 conv, 128 -> 64 channels -> catpad2[64:128], fp8
                # DoubleRow: 4 paired taps + 1 single.  The weights sit in
                # lhsT columns 64:128 (0:64 zero), so the conv lands
                # directly on psum partitions 64:128 and the relu writes
                # catpad2[64:128] straight from PSUM.
                ps = pspool.tile([128, 4, 128], F32, tag="ps", name=f"psf2_{cc}")
                for j, (base, partner) in enumerate(F2_PAIRS):
                    nc.tensor.matmul(
                        ps,
                        wf2s[:, 2 * j : 2 * j + 2, :],
                        _pair_rhs(flo3, cc, base, partner),
                        start=(j == 0),
                        stop=False,
                        perf_mode=DR,
                    )
                dh, dw = F2_SINGLE
                nc.tensor.matmul(
                    ps,
                    wf2s[:, 8, :],
                    flo3[:, dw, cc + 1 + dh : cc + 5 + dh, :],
                    start=False,
                    stop=True,
                )
                nc.scalar.activation(
                    catpad2[64:128, cc + 1 : cc + 5, 1 : 1 + W],
                    ps[64:128],
                    RELU,
                    bias=bs[64:128, 5:6],
                )

            def emit_c1_dma(rr):
                ct = spool.tile([128, 3, 4, 128], BF16, tag="corr", name=f"ct_{rr}")
                nc.sync.dma_start(out=ct, in_=corr_r[:, :, rr : rr + 4, :])
                return ct

            def emit_c1_mm(rr, ct):
                ps0 = pspool.tile([128, 4, 128], F32, tag="ps", name=f"psc1a_{rr}")
                ps1 = pspool.tile([128, 4, 128], F32, tag="ps", name=f"psc1b_{rr}")
                for kt in range(3):
                    nc.tensor.matmul(
                        ps0, wc1s[:, kt, 0:128], ct[:, kt], start=(kt == 0), stop=(kt == 2)
                    )
                    nc.tensor.matmul(
                        ps1, wc1s[:, kt, 128:256], ct[:, kt], start=(kt == 0), stop=(kt == 2)
                    )
                r = rr + 2
                nc.scalar.activation(cor1a[:, r : r + 4, 1 : 1 + W], ps0, RELU, bias=bs[:, 0:1])
                nc.scalar.activation(cor1b[:, r : r + 4, 1 : 1 + W], ps1, RELU, bias=bs[:, 1:2])

            def emit_c2(cc):
                ps0 = pspool.tile([128, 4, 128], F32, tag="ps", name=f"psc2a_{cc}")
                ps1 = pspool.tile([128, 4, 128], F32, tag="ps", name=f"psc2b_{cc}")
                k = 0
                for kt, src_ in enumerate((cor1a, cor1b)):
                    for dh in range(3):
                        for dw in range(3):
                            i = cc + 1 + dh
                            rhs = src_[:, i : i + 4, dw : dw + 128]
                            tap = 3 * dh + dw
                            nc.tensor.matmul(
                                ps0, wc2s[:, tap, kt, 0:128], rhs, start=(k == 0), stop=(k == 17)
                            )
                            nc.tensor.matmul(
                                ps1,
                                wc2s[:, tap, kt, 128:256],
                                rhs,
                                start=(k == 0),
                                stop=(k == 17),
                            )
                            k += 1
                r = cc + 1
                nc.scalar.activation(catpad1[:, r : r + 4, 1 : 1 + W], ps0, RELU, bias=bs[:, 2:3])
                nc.scalar.activation(
                    catpad2[0:64, r : r + 4, 1 : 1 + W], ps1[0:64], RELU, bias=bs[0:64, 3:4]
                )

            def emit_o(oo, split=False):
                ps = pspool.tile([128, 4, 128], F32, tag="ps", name=f"pso_{oo}")
                k = 0
                for kt, src_ in enumerate((catpad1, catpad2)):
                    for dh in range(3):
                        for dw in range(3):
                            i = oo + dh
                            nc.tensor.matmul(
                                ps[0:126],
                                wos[:, kt, 3 * dh + dw, :],
                                src_[:, i : i + 4, dw : dw + 128],
                                start=(k == 0),
                                stop=(k == 17),
                            )
                            k += 1
                ob = opool.tile([128, 4, 128], F32, tag="ob", name=f"ob_{oo}")
                if split:
                    # last block: 2-row ACT halves so the final out DMA
                    # starts ~0.3us earlier (it ends the kernel).
                    for h in range(2):
                        nc.scalar.activation(
                            ob[0:126, 2 * h : 2 * h + 2],
                            ps[0:126, 2 * h : 2 * h + 2],
                            RELU,
                            bias=bs[0:126, 6:7],
                        )
                        nc.sync.dma_start(
                            out=out[:, oo + 2 * h : oo + 2 * h + 2, :],
                            in_=ob[0:126, 2 * h : 2 * h + 2],
                        )
                else:
                    nc.scalar.activation(ob[0:126], ps[0:126], RELU, bias=bs[0:126, 6:7])
                    nc.sync.dma_start(out=out[:, oo : oo + 4, :], in_=ob[0:126])

            # --- the streamed pipeline.  c1 has only 6 matmuls per block
            # against ~2us of corr DMA, so on its own it starves the PE;
            # staggering f2 (5 matmuls), c2 (36) and o (18) behind it
            # keeps the PE dense while corr streams.
            # Prologue ordering principle: the Tensor queue is in-order,
            # so ACT/DVE-gated work (f2 reads flo3 = f1 relu + DVE plane
            # copies, two hops deep) must sit BEHIND DMA-fed work (f1
            # from stack2, c1 from the ct stream) — interleaving c1
            # between the early f1/f2 blocks keeps the PE dense while
            # the scalar/vector queues drain their startup backlog.
            cts_q = {}
            for j in range(4):
                cts_q[j] = emit_c1_dma(4 * j)
            for j in range(4):
                emit_f1(4 * j)
            emit_c1_mm(0, cts_q.pop(0))
            emit_f1(16)
            emit_f1(20)
            emit_c1_mm(4, cts_q.pop(1))
            emit_c1_mm(8, cts_q.pop(2))
            emit_f2(0)
            emit_f2(4)
            emit_c2(0)
            # o trails c2 by 4 blocks in steady state (so it never waits
            # on the same-iteration c2 drain), but the tail is compressed:
            # o(22) rides with o(21) one iteration early, trading a ~0.6us
            # ACT wait for a whole 3.9us single-stream drain iteration.
            for idx in range(3, NBLK + 3):
                if idx + 1 < NBLK:
                    cts_q[idx + 1] = emit_c1_dma(4 * (idx + 1))
                if idx + 3 < NBLK:
                    emit_f1(4 * (idx + 3))
                if idx < NBLK:
                    emit_c1_mm(4 * idx, cts_q.pop(idx))
                if idx - 1 < NBLK:
                    emit_f2(4 * (idx - 1))
                if idx - 2 < NBLK:
                    emit_c2(4 * (idx - 2))
                if 0 <= idx - 4 < NBLK - 2:
                    emit_o(4 * (idx - 4))
                if idx == NBLK + 1:
                    emit_o(4 * (NBLK - 2))
                if idx == NBLK + 2:
                    emit_o(4 * (NBLK - 1), split=True)
    nc.compile()
    return nc


def pack_params(wc1, bc1, wc2, bc2, wf1, bf1, wf2, bf2, wo, bo):
    """Host-side repack of OIHW conv weights into the lhsT layouts the
    kernel's matmuls read ([K partitions, ..., M])."""
    f = np.float32
    wc1p = np.zeros((128, 3, 256), f)
    w = wc1[:, :, 0, 0]  # [256, 324]
    for kt in range(3):
        kk = min(128, CIN_CORR - kt * 128)
        wc1p[0:kk, kt, :] = w[:, kt * 128 : kt * 128 + kk].T
    wc2p = np.zeros((128, 9, 2, 256), f)
    for dh in range(3):
        for dw in range(3):
            for kt in range(2):
                wc2p[:, 3 * dh + dw, kt, 0:128] = wc2[0:128, kt * 128 : kt * 128 + 128, dh, dw].T
                wc2p[:, 3 * dh + dw, kt, 128:192] = wc2[128:192, kt * 128 : kt * 128 + 128, dh, dw].T
    wf1p = np.zeros((128, 128), f)
    for cin in range(2):
        for dh in range(7):
            for dw in range(7):
                wf1p[cin * 49 + dh * 7 + dw, :] = wf1[:, cin, dh, dw]
    # f2 weights in fp8, DoubleRow pair order: slots 2j/2j+1 hold the
    # base/partner taps of F2_PAIRS[j], slot 8 the single tap.  couts sit
    # in lhsT columns 64:128 (psum partition targeting).
    wf2p = np.zeros((128, 9, 128), f)
    for j, (base, partner) in enumerate(F2_PAIRS):
        for s, (dh, dw) in enumerate((base, partner)):
            wf2p[:, 2 * j + s, 64:128] = wf2[:, :, dh, dw].T
    dh, dw = F2_SINGLE
    wf2p[:, 8, 64:128] = wf2[:, :, dh, dw].T
    wop = np.zeros((128, 2, 9, 126), f)
    for dh in range(3):
        for dw in range(3):
            tap = 3 * dh + dw
            wop[:, 0, tap, :] = wo[:, 0:128, dh, dw].T
            wop[0:64, 1, tap, :] = wo[:, 128:192, dh, dw].T
            wop[64:128, 1, tap, :] = wo[:, 192:256, dh, dw].T
    biasp = np.zeros((128, 8), f)
    biasp[:, 0] = bc1[0:128]
    biasp[:, 1] = bc1[128:256]
    biasp[:, 2] = bc2[0:128]
    biasp[0:64, 3] = bc2[128:192]
    biasp[:, 4] = bf1
    biasp[64:128, 5] = bf2
    biasp[0:126, 6] = bo
    return {
        "wc1p": wc1p.astype(NPBF16),
        "wc2p": wc2p.astype(NPBF16),
        "wf1p": wf1p.astype(NPBF16),
        "wf2p": wf2p.astype(NPF8),
        "wop": wop.astype(NPBF16),
        "biasp": biasp,
    }


def build_stackh(flow_b):
    """Full f1 im2col: [98, H+4, 128], partition cin*49+dh*7+dw holds the
    zero-padded (pad 3) flow image shifted by (dh, dw); row i <-> f1 output
    row i-2."""
    fz = np.zeros((2, H + 10, W + 6), NPBF16)
    fz[:, 5 : 5 + H, 3 : 3 + W] = flow_b
    s = np.zeros((128, H + 4, 128), NPBF16)
    for cin in range(2):
        for dh in range(7):
            for dw in range(7):
                # output row r (= buffer row r+2) col c reads fz row r+dh+2, col c+dw
                s[cin * 49 + dh * 7 + dw] = fz[cin, dh : dh + H + 4, dw : dw + 128]
    return s


_MODULE = None


def _get_module():
    global _MODULE
    if _MODULE is None:
        _MODULE = build_module()
    return _MODULE


def make_in_maps(**inputs):
    a = {
        k: np.ascontiguousarray(np.asarray(v), dtype=np.float32)
        for k, v in inputs.items()
    }
    packed = pack_params(
        a["wc1"], a["bc1"], a["wc2"], a["bc2"], a["wf1"], a["bf1"],
        a["wf2"], a["bf2"], a["wo"], a["bo"],
    )
    corr_bf = a["corr"].astype(NPBF16)
    in_maps = []
    for b in range(8):
        m = dict(packed)
        cp = np.zeros((384, H, W), NPBF16)
        cp[0:CIN_CORR] = corr_bf[b]
        m["corr"] = cp
        m["stackh"] = build_stackh(a["flow"][b])
        in_maps.append(m)
    return in_maps, a["flow"]


def assemble_output(results, flow):
    out = np.empty((8, 128, H, W), np.float32)
    for b in range(8):
        out[b, :126] = results[b]["out"]
        out[b, 126:] = flow[b]
    return out


def run(trace=False, **inputs):
    in_maps, flow = make_in_maps(**inputs)
    nc = _get_module()
    res = run_bass_kernel_spmd(nc, in_maps, core_ids=list(range(8)), trace=trace)
    return assemble_output(res.results, flow), res


def kernel(**inputs):
    out, _ = run(trace=False, **inputs)
    return out


# revision 16
# speedup vs baseline: 1.0162x; 1.0162x over previous
"""Trainium2 Bass kernel for BasicMotionEncoder (RAFT motion encoder).

Network (all stride-1 convs, NCHW, fp32 in/out):
    cor  = relu(conv1x1(corr, wc1, bc1))          # [B,256,H,W]
    cor  = relu(conv3x3(cor,  wc2, bc2, pad 1))   # [B,192,H,W]
    flo  = relu(conv7x7(flow, wf1, bf1, pad 3))   # [B,128,H,W]
    flo  = relu(conv3x3(flo,  wf2, bf2, pad 1))   # [B,64,H,W]
    out  = relu(conv3x3(cat(cor,flo), wo, bo, 1)) # [B,126,H,W]
    return cat(out, flow)                         # [B,128,H,W]

Sharding: pure data parallel, one image per NeuronCore (B=8, 8 cores).
Each core streams its image through the 5-layer pipeline in 24 4-row
blocks with all intermediate activations resident in SBUF at full image
height (no pass/halo recompute); only corr is streamed in and the
126-channel output written back.  Convs are PE matmuls with channels on
the partition dim: for each tap the shifted input window is a strided AP
into a zero-padded SBUF image, accumulated in PSUM over taps and k-tiles
(bf16 operands, fp32 PSUM — bf16 is 1 row/cycle through the PE like
fp32r but halves LDWEIGHTS + operand SBUF traffic; rel-err ~4e-3 vs the
2e-2 budget).  All M<=64 weights are padded to M=128: half-width
(col_grp=h0) LDWEIGHTS mixed with full-width ones break weight-load/
matmul overlap and cost ~110ns per matmul, while matmul time depends
only on the free dim, not M.  The final concat of `flow` into channels
126:128 happens on the host.

f2 runs in fp8(e4m3) MatmulPerfMode.DoubleRow: each DR matmul contracts
TWO taps (lhsT [K,2,M], rhs [K,2,N]) in the same 512-cycle stream,
taking f2 from 9 matmuls/block to 4 paired + 1 single (4608c -> 2560c,
~21us).  End-to-end rel-err with f2-only fp8 is ~9e-3 (sim-verified)
vs the 2e-2 budget.  The PE ifmap walker only supports the pair dim
plus ONE more (merged) dim, so the paired windows must be contiguous:
f1's output is stored as THREE 128-wide column-shifted planes
flo3[plane dw][c] = flo[c+dw-1] (f1's ACT writes the center plane, the
otherwise-idle DVE copies the +-1 shifted planes), making every 3x3-tap
window a flat 512-run and the pair AP [K, (delta,2), (1,512)].
"""

import ml_dtypes
import numpy as np

import bass_rust
import concourse.mybir as mybir
import concourse.tile as tile
from concourse import bacc
from concourse.bass_utils import run_bass_kernel_spmd

H, W = 96, 128
CIN_CORR = 324
WP = W + 2  # pad-1 padded row width (3x3 convs)
NBLK = H // 4  # 4-row output blocks streamed through the pipeline
F32 = mybir.dt.float32
BF16 = mybir.dt.bfloat16
F8 = mybir.dt.float8e4
NPBF16 = ml_dtypes.bfloat16
NPF8 = ml_dtypes.float8_e4m3fn
RELU = mybir.ActivationFunctionType.Relu
COPY = mybir.ActivationFunctionType.Copy
DR = mybir.MatmulPerfMode.DoubleRow

ZELEMS = 512  # zeros tile length (>= 2 x buffer rows; 512 for warmup rhs)
# Zero-weight warmup matmuls: bridge PE start (~8.1us) to the first
# stack-chunk DMA arrival (~12.5-13us) with NO gap, so the 1.2->2.4GHz
# ramp (3us continuous) completes on useless work and the real stream
# starts at full clock.  Any PE gap resets the ramp: ~3us of 427ns
# matmuls.  16 x 512-free: ~7 at 427ns (ramping) + ~9 at 213ns.
NWARM = 16

# flo3 plane geometry: [128, 3, H+4, 128]; plane stride in elements.
FLO_ROWS = H + 4
FLO_PLANE = FLO_ROWS * 128

# f2 DoubleRow tap pairing: taps (dh,dw); pair = (base_tap, partner_tap)
# with constant AP delta = (dw'-dw)*FLO_PLANE + (dh'-dh)*128.  Windows
# for tap (dh,dw) on block cc: flo3[:, dw, cc+1+dh : cc+5+dh, :].
F2_PAIRS = [((0, 0), (0, 1)), ((1, 0), (0, 2)), ((1, 1), (1, 2)), ((2, 0), (2, 1))]
F2_SINGLE = (2, 2)

# Row maps.  cor1 buffers: buffer row r holds image row r-2 (rows 1
# and H+2 are the zero-pad rows the edge taps read; 0 and H+3 unused).
# flo3 planes use the same row map at width 128 (plane dw pre-shifted
# by dw-1 columns).  catpad buffers: buffer row r holds cat row r-1
# (rows 0 and H+1 zero).


def _zero_borders(nc, buf, zrows):
    """Zero the conv-padding bytes of a padded image buffer with vector-
    engine memsets: cols {0,1} and {128,129} of every row (cols 1/128 are
    interior and overwritten by the relu writes that follow), plus the
    vertical-padding zero rows the edge taps read.  These ride the
    otherwise-idle DVE queue: as scalar-ACT copies they serialized in
    front of the first f1 relu and stalled the f2 pipeline ~2-3us."""
    for off in (0, W):
        nc.vector.memset(buf[:, :, off : off + 2], 0.0)
    for zrow in zrows:
        nc.vector.memset(buf[:, zrow : zrow + 1, :], 0.0)


def _pair_rhs(flo3, cc, base, partner):
    """rhs AP for a DoubleRow tap pair: the base tap's contiguous
    [4x128] window with an inserted (delta, 2) pair dim."""
    (dh, dw), (dh2, dw2) = base, partner
    delta = (dw2 - dw) * FLO_PLANE + (dh2 - dh) * 128
    rhs = flo3[:, dw, cc + 1 + dh : cc + 5 + dh, :].copy()
    ap = rhs.ap.to_list()  # [(pstride,128), (128,4), (1,128)]
    rhs.ap = bass_rust.VecI64Pair([ap[0], (delta, 2), (1, 512)])
    return rhs


def build_module():
    nc = bacc.Bacc(trn_type="TRN2", target_bir_lowering=False)
    # corr is zero-padded to 384 channels on the host so the three c1
    # k-tiles are a single DMA and a uniform K=128 contraction.
    corr = nc.dram_tensor("corr", [384, H, W], BF16, kind="ExternalInput").ap()
    corr_r = corr.rearrange("(kt p) h w -> p kt h w", kt=3)
    # f1 im2col K is padded 98 -> 128: partial-K LDWEIGHTS mixed with
    # full-K ones cost ~160ns per f1 matmul (same penalty class as
    # half-width col_grp loads).
    stackh = nc.dram_tensor("stackh", [128, H + 4, 128], BF16, kind="ExternalInput").ap()
    wc1p = nc.dram_tensor("wc1p", [128, 3, 256], BF16, kind="ExternalInput").ap()
    # wc2 packed WITHOUT the 192->256 M-pad: the ps1 matmul reads cols
    # 64:192 (couts 64:192, full 128-wide LDWEIGHTS) and its relu keeps
    # psum partitions 64:128 (couts 128:192) -> catpad2[64:128].  Saves
    # 0.59MB of prologue DMA, which is bandwidth-bound.
    wc2p = nc.dram_tensor("wc2p", [128, 9, 2, 192], BF16, kind="ExternalInput").ap()
    wf1p = nc.dram_tensor("wf1p", [128, 128], BF16, kind="ExternalInput").ap()
    wf2p = nc.dram_tensor("wf2p", [128, 9, 128], F8, kind="ExternalInput").ap()
    wop = nc.dram_tensor("wop", [128, 2, 9, 126], BF16, kind="ExternalInput").ap()
    biasp = nc.dram_tensor("biasp", [128, 8], F32, kind="ExternalInput").ap()
    out = nc.dram_tensor("out", [126, H, W], F32, kind="ExternalOutput").ap()

    with tile.TileContext(nc) as tc:
        with (
            tc.tile_pool(name="wpool", bufs=1) as wpool,
            tc.tile_pool(name="pspool", space="PSUM", bufs=8) as pspool,
            tc.tile_pool(name="spool", bufs=4) as spool,
            tc.tile_pool(name="opool", bufs=3) as opool,
        ):
            wc1s = wpool.tile([128, 3, 256], BF16, name="wc1s")
            wc2s = wpool.tile([128, 9, 2, 192], BF16, name="wc2s")
            wf1s = wpool.tile([128, 128], BF16, name="wf1s")
            wf2s = wpool.tile([128, 9, 128], F8, name="wf2s")
            wos = wpool.tile([128, 2, 9, 126], BF16, name="wos")
            bs = wpool.tile([128, 8], F32, name="bs")
            zsb = wpool.tile([128, ZELEMS], BF16, name="zsb")
            scr = wpool.tile([128, 1], F32, name="scr")
            # full-height intermediates, written once per row (no halo)
            stack2 = wpool.tile([128, H + 4, 128], BF16, name="stack2")
            flo3 = wpool.tile([128, 3, FLO_ROWS, 128], F8, name="flo3")
            cor1a = wpool.tile([128, H + 4, WP], BF16, name="cor1a")
            cor1b = wpool.tile([128, H + 4, WP], BF16, name="cor1b")
            catpad1 = wpool.tile([128, H + 2, WP], BF16, name="catpad1")
            catpad2 = wpool.tile([128, H + 2, WP], BF16, name="catpad2")

            # --- setup.  The zeros tile is a vector-engine memset (a DMA
            # through the Activation-queue DGE lands ~14us late and the
            # zero-border ACTs -> f1 relu -> f2 chain all wait on it).
            # The prologue (8-20us) is DMA-BANDWIDTH-bound: the 16 DMA
            # engines fair-share across the sync/gpsimd/scalar queues, so
            # every byte of early weight traffic delays the corr (ct)
            # stream the c1 matmuls wait on.  Early set is the minimum:
            # sync = stack[0:28] + ct0..ct3 (the prologue's c1 food),
            # gpsimd = bs/wf2s/wc1s then the two wc2s halves, scalar =
            # wf1s only (so the f1 relus start ~14us, not behind big
            # triggers).  wos and the remaining stack chunks ride later
            # (gpsimd tail / sync in-loop at idx 3/6/9) -- they are not
            # needed before ~30-100us.
            nc.vector.memset(zsb, 0.0)
            nc.sync.dma_start(out=stack2[:, 0:28, :], in_=stackh[:, 0:28, :])
            # Zero-weight warmup matmuls bridge the wait for the first
            # stack DMA: the PE needs ~3us of continuous execution to
            # ramp 1.2GHz -> 2.4GHz, so by the time real work arrives
            # the clock is at full speed.
            psd = pspool.tile([128, 4, 128], F32, tag="ps", name="psdum")
            zv1 = zsb.rearrange("p (a b) -> p a b", a=1)
            for i in range(NWARM):
                nc.tensor.matmul(
                    psd, zsb[:, 0:128], zv1, start=(i == 0), stop=(i == NWARM - 1)
                )
            nc.scalar.dma_start(out=wf1s, in_=wf1p)
            nc.gpsimd.dma_start(out=bs, in_=biasp)
            nc.gpsimd.dma_start(out=wf2s, in_=wf2p)
            nc.gpsimd.dma_start(out=wc1s, in_=wc1p)
            # wc2s in two k-halves: c2(0) only needs kt=0 for its first
            # 18 matmuls, so the first half landing earlier removes the
            # c2(0) stall
            nc.gpsimd.dma_start(out=wc2s[:, :, 0, :], in_=wc2p[:, :, 0, :])
            nc.gpsimd.dma_start(out=wc2s[:, :, 1, :], in_=wc2p[:, :, 1, :])
            nc.gpsimd.dma_start(out=wos, in_=wop)
            # prewarm the Relu activation table off the critical path
            nc.scalar.activation(scr, zsb[:, 0:1], RELU)
            # flo3 zero regions: vertical pad rows 1 and H+2 (all three
            # planes), plane0 col 0 (left pad), plane2 col 127 (right
            # pad).  Interior rows/cols are written by ACT/DVE each block.
            for zrow in (1, H + 2):
                nc.vector.memset(flo3[:, :, zrow : zrow + 1, :], 0.0)
            nc.vector.memset(flo3[:, 0, :, 0:1], 0.0)
            nc.vector.memset(flo3[:, 2, :, 127:128], 0.0)
            _zero_borders(nc, cor1a, (1, H + 2))
            _zero_borders(nc, cor1b, (1, H + 2))
            _zero_borders(nc, catpad1, (0, H + 1))
            _zero_borders(nc, catpad2, (0, H + 1))

            def emit_f1(rr):
                # 7x7 conv, 2 -> 128 channels.  Input is a full host-side
                # im2col: stack2 partition cin*49+dh*7+dw holds the
                # zero-padded flow image shifted by (dh, dw), so one K=98
                # matmul computes a whole block.  The relu writes the fp8
                # center plane of flo3; the DVE then copies the +-1
                # column-shifted planes the f2 DoubleRow windows read.
                ps = pspool.tile([128, 4, 128], F32, tag="ps", name=f"psf1_{rr}")
                i = rr + 2
                nc.tensor.matmul(ps, wf1s, stack2[:, i : i + 4, :], start=True, stop=True)
                nc.scalar.activation(flo3[:, 1, i : i + 4, :], ps, RELU, bias=bs[:, 4:5])
                nc.vector.tensor_copy(
                    flo3[:, 0, i : i + 4, 1:128], flo3[:, 1, i : i + 4, 0:127]
                )
                nc.vector.tensor_copy(
                    flo3[:, 2, i : i + 4, 0:127], flo3[:, 1, i : i + 4, 1:128]
                )

            def emit_f2(cc):
                # 3x3 conv, 128 -> 64 channels -> catpad2[0:64], fp8
                # DoubleRow: 4 paired taps + 1 single.  The weights sit in
                # lhsT columns 0:64 (64:128 zero), so the conv lands
                # directly on psum partitions 0:64 and the relu writes
                # catpad2[0:64] straight from PSUM.  (catpad2 partition
                # map: 0:64 = f2 couts = cat ch 192:256, 64:128 = c2
                # couts 128:192; wop kt1 rows match.)
                ps = pspool.tile([128, 4, 128], F32, tag="ps", name=f"psf2_{cc}")
                for j, (base, partner) in enumerate(F2_PAIRS):
                    nc.tensor.matmul(
                        ps,
                        wf2s[:, 2 * j : 2 * j + 2, :],
                        _pair_rhs(flo3, cc, base, partner),
                        start=(j == 0),
                        stop=False,
                        perf_mode=DR,
                    )
                dh, dw = F2_SINGLE
                nc.tensor.matmul(
                    ps,
                    wf2s[:, 8, :],
                    flo3[:, dw, cc + 1 + dh : cc + 5 + dh, :],
                    start=False,
                    stop=True,
                )
                nc.scalar.activation(
                    catpad2[0:64, cc + 1 : cc + 5, 1 : 1 + W],
                    ps[0:64],
                    RELU,
                    bias=bs[0:64, 5:6],
                )

            def emit_c1_dma(rr):
                ct = spool.tile([128, 3, 4, 128], BF16, tag="corr", name=f"ct_{rr}")
                nc.sync.dma_start(out=ct, in_=corr_r[:, :, rr : rr + 4, :])
                return ct

            def emit_c1_mm(rr, ct):
                ps0 = pspool.tile([128, 4, 128], F32, tag="ps", name=f"psc1a_{rr}")
                ps1 = pspool.tile([128, 4, 128], F32, tag="ps", name=f"psc1b_{rr}")
                for kt in range(3):
                    nc.tensor.matmul(
                        ps0, wc1s[:, kt, 0:128], ct[:, kt], start=(kt == 0), stop=(kt == 2)
                    )
                    nc.tensor.matmul(
                        ps1, wc1s[:, kt, 128:256], ct[:, kt], start=(kt == 0), stop=(kt == 2)
                    )
                r = rr + 2
                nc.scalar.activation(cor1a[:, r : r + 4, 1 : 1 + W], ps0, RELU, bias=bs[:, 0:1])
                nc.scalar.activation(cor1b[:, r : r + 4, 1 : 1 + W], ps1, RELU, bias=bs[:, 1:2])

            def emit_c2(cc):
                # ps1 reads lhsT cols 64:192 (couts 64:192, full-width
                # LDWEIGHTS, no M-pad DMA) and keeps psum partitions
                # 64:128 = couts 128:192 -> catpad2[64:128].
                ps0 = pspool.tile([128, 4, 128], F32, tag="ps", name=f"psc2a_{cc}")
                ps1 = pspool.tile([128, 4, 128], F32, tag="ps", name=f"psc2b_{cc}")
                k = 0
                for kt, src_ in enumerate((cor1a, cor1b)):
                    for dh in range(3):
                        for dw in range(3):
                            i = cc + 1 + dh
                            rhs = src_[:, i : i + 4, dw : dw + 128]
                            tap = 3 * dh + dw
                            nc.tensor.matmul(
                                ps0, wc2s[:, tap, kt, 0:128], rhs, start=(k == 0), stop=(k == 17)
                            )
                            nc.tensor.matmul(
                                ps1,
                                wc2s[:, tap, kt, 64:192],
                                rhs,
                                start=(k == 0),
                                stop=(k == 17),
                            )
                            k += 1
                r = cc + 1
                nc.scalar.activation(catpad1[:, r : r + 4, 1 : 1 + W], ps0, RELU, bias=bs[:, 2:3])
                nc.scalar.activation(
                    catpad2[64:128, r : r + 4, 1 : 1 + W],
                    ps1[64:128],
                    RELU,
                    bias=bs[64:128, 3:4],
                )

            def emit_o(oo, split=False):
                ps = pspool.tile([128, 4, 128], F32, tag="ps", name=f"pso_{oo}")
                k = 0
                for kt, src_ in enumerate((catpad1, catpad2)):
                    for dh in range(3):
                        for dw in range(3):
                            i = oo + dh
                            nc.tensor.matmul(
                                ps[0:126],
                                wos[:, kt, 3 * dh + dw, :],
                                src_[:, i : i + 4, dw : dw + 128],
                                start=(k == 0),
                                stop=(k == 17),
                            )
                            k += 1
                ob = opool.tile([128, 4, 128], F32, tag="ob", name=f"ob_{oo}")
                if split:
                    # last block: 2-row ACT halves so the final out DMA
                    # starts ~0.3us earlier (it ends the kernel).
                    for h in range(2):
                        nc.scalar.activation(
                            ob[0:126, 2 * h : 2 * h + 2],
                            ps[0:126, 2 * h : 2 * h + 2],
                            RELU,
                            bias=bs[0:126, 6:7],
                        )
                        nc.sync.dma_start(
                            out=out[:, oo + 2 * h : oo + 2 * h + 2, :],
                            in_=ob[0:126, 2 * h : 2 * h + 2],
                        )
                else:
                    nc.scalar.activation(ob[0:126], ps[0:126], RELU, bias=bs[0:126, 6:7])
                    nc.sync.dma_start(out=out[:, oo : oo + 4, :], in_=ob[0:126])

            # --- the streamed pipeline.  c1 has only 6 matmuls per block
            # against ~2us of corr DMA, so on its own it starves the PE;
            # staggering f2 (5 matmuls), c2 (36) and o (18) behind it
            # keeps the PE dense while corr streams.
            # Prologue ordering principle: the Tensor queue is in-order,
            # so ACT/DVE-gated work (f2 reads flo3 = f1 relu + DVE plane
            # copies, two hops deep) must sit BEHIND DMA-fed work (f1
            # from stack2, c1 from the ct stream) — interleaving c1
            # between the early f1/f2 blocks keeps the PE dense while
            # the scalar/vector queues drain their startup backlog.
            cts_q = {}
            for j in range(4):
                cts_q[j] = emit_c1_dma(4 * j)
            for j in range(4):
                emit_f1(4 * j)
            emit_c1_mm(0, cts_q.pop(0))
            emit_f1(16)
            emit_f1(20)
            emit_c1_mm(4, cts_q.pop(1))
            emit_c1_mm(8, cts_q.pop(2))
            emit_f2(0)
            emit_f2(4)
            emit_c2(0)
            # o trails c2 by 4 blocks in steady state (so it never waits
            # on the same-iteration c2 drain), but the tail is compressed:
            # o(22) rides with o(21) one iteration early, trading a ~0.6us
            # ACT wait for a whole 3.9us single-stream drain iteration.
            for idx in range(3, NBLK + 3):
                if idx + 1 < NBLK:
                    cts_q[idx + 1] = emit_c1_dma(4 * (idx + 1))
                # remaining stack chunks ride the (now slack) sync queue
                # well ahead of their first f1 consumer
                if idx == 3:
                    nc.sync.dma_start(out=stack2[:, 28:52, :], in_=stackh[:, 28:52, :])
                elif idx == 6:
                    nc.sync.dma_start(out=stack2[:, 52:76, :], in_=stackh[:, 52:76, :])
                elif idx == 9:
                    nc.sync.dma_start(out=stack2[:, 76:100, :], in_=stackh[:, 76:100, :])
                if idx + 3 < NBLK:
                    emit_f1(4 * (idx + 3))
                if idx < NBLK:
                    emit_c1_mm(4 * idx, cts_q.pop(idx))
                if idx - 1 < NBLK:
                    emit_f2(4 * (idx - 1))
                if idx - 2 < NBLK:
                    emit_c2(4 * (idx - 2))
                if 0 <= idx - 4 < NBLK - 2:
                    emit_o(4 * (idx - 4))
                if idx == NBLK + 1:
                    emit_o(4 * (NBLK - 2))
                if idx == NBLK + 2:
                    emit_o(4 * (NBLK - 1), split=True)
    nc.compile()
    return nc


def pack_params(wc1, bc1, wc2, bc2, wf1, bf1, wf2, bf2, wo, bo):
    """Host-side repack of OIHW conv weights into the lhsT layouts the
    kernel's matmuls read ([K partitions, ..., M])."""
    f = np.float32
    wc1p = np.zeros((128, 3, 256), f)
    w = wc1[:, :, 0, 0]  # [256, 324]
    for kt in range(3):
        kk = min(128, CIN_CORR - kt * 128)
        wc1p[0:kk, kt, :] = w[:, kt * 128 : kt * 128 + kk].T
    wc2p = np.zeros((128, 9, 2, 192), f)
    for dh in range(3):
        for dw in range(3):
            for kt in range(2):
                wc2p[:, 3 * dh + dw, kt, :] = wc2[:, kt * 128 : kt * 128 + 128, dh, dw].T
    wf1p = np.zeros((128, 128), f)
    for cin in range(2):
        for dh in range(7):
            for dw in range(7):
                wf1p[cin * 49 + dh * 7 + dw, :] = wf1[:, cin, dh, dw]
    # f2 weights in fp8, DoubleRow pair order: slots 2j/2j+1 hold the
    # base/partner taps of F2_PAIRS[j], slot 8 the single tap.  couts sit
    # in lhsT columns 0:64 (psum partition targeting -> catpad2[0:64]).
    wf2p = np.zeros((128, 9, 128), f)
    for j, (base, partner) in enumerate(F2_PAIRS):
        for s, (dh, dw) in enumerate((base, partner)):
            wf2p[:, 2 * j + s, 0:64] = wf2[:, :, dh, dw].T
    dh, dw = F2_SINGLE
    wf2p[:, 8, 0:64] = wf2[:, :, dh, dw].T
    # o kt1 rows follow the catpad2 partition map: 0:64 = f2 couts
    # (cat ch 192:256), 64:128 = c2 couts 128:192.
    wop = np.zeros((128, 2, 9, 126), f)
    for dh in range(3):
        for dw in range(3):
            tap = 3 * dh + dw
            wop[:, 0, tap, :] = wo[:, 0:128, dh, dw].T
            wop[0:64, 1, tap, :] = wo[:, 192:256, dh, dw].T
            wop[64:128, 1, tap, :] = wo[:, 128:192, dh, dw].T
    biasp = np.zeros((128, 8), f)
    biasp[:, 0] = bc1[0:128]
    biasp[:, 1] = bc1[128:256]
    biasp[:, 2] = bc2[0:128]
    biasp[64:128, 3] = bc2[128:192]
    biasp[:, 4] = bf1
    biasp[0:64, 5] = bf2
    biasp[0:126, 6] = bo
    return {
        "wc1p": wc1p.astype(NPBF16),
        "wc2p": wc2p.astype(NPBF16),
        "wf1p": wf1p.astype(NPBF16),
        "wf2p": wf2p.astype(NPF8),
        "wop": wop.astype(NPBF16),
        "biasp": biasp,
    }


def build_stackh(flow_b):
    """Full f1 im2col: [98, H+4, 128], partition cin*49+dh*7+dw holds the
    zero-padded (pad 3) flow image shifted by (dh, dw); row i <-> f1 output
    row i-2."""
    fz = np.zeros((2, H + 10, W + 6), NPBF16)
    fz[:, 5 : 5 + H, 3 : 3 + W] = flow_b
    s = np.zeros((128, H + 4, 128), NPBF16)
    for cin in range(2):
        for dh in range(7):
            for dw in range(7):
                # output row r (= buffer row r+2) col c reads fz row r+dh+2, col c+dw
                s[cin * 49 + dh * 7 + dw] = fz[cin, dh : dh + H + 4, dw : dw + 128]
    return s


_MODULE = None


def _get_module():
    global _MODULE
    if _MODULE is None:
        _MODULE = build_module()
    return _MODULE


def make_in_maps(**inputs):
    a = {
        k: np.ascontiguousarray(np.asarray(v), dtype=np.float32)
        for k, v in inputs.items()
    }
    packed = pack_params(
        a["wc1"], a["bc1"], a["wc2"], a["bc2"], a["wf1"], a["bf1"],
        a["wf2"], a["bf2"], a["wo"], a["bo"],
    )
    corr_bf = a["corr"].astype(NPBF16)
    in_maps = []
    for b in range(8):
        m = dict(packed)
        cp = np.zeros((384, H, W), NPBF16)
        cp[0:CIN_CORR] = corr_bf[b]
        m["corr"] = cp
        m["stackh"] = build_stackh(a["flow"][b])
        in_maps.append(m)
    return in_maps, a["flow"]


def assemble_output(results, flow):
    out = np.empty((8, 128, H, W), np.float32)
    for b in range(8):
        out[b, :126] = results[b]["out"]
        out[b, 126:] = flow[b]
    return out


def run(trace=False, **inputs):
    in_maps, flow = make_in_maps(**inputs)
    nc = _get_module()
    res = run_bass_kernel_spmd(nc, in_maps, core_ids=list(range(8)), trace=trace)
    return assemble_output(res.results, flow), res


def kernel(**inputs):
    out, _ = run(trace=False, **inputs)
    return out


# revision 20
# speedup vs baseline: 1.0294x; 1.0130x over previous
"""Trainium2 Bass kernel for BasicMotionEncoder (RAFT motion encoder).

Network (all stride-1 convs, NCHW, fp32 in/out):
    cor  = relu(conv1x1(corr, wc1, bc1))          # [B,256,H,W]
    cor  = relu(conv3x3(cor,  wc2, bc2, pad 1))   # [B,192,H,W]
    flo  = relu(conv7x7(flow, wf1, bf1, pad 3))   # [B,128,H,W]
    flo  = relu(conv3x3(flo,  wf2, bf2, pad 1))   # [B,64,H,W]
    out  = relu(conv3x3(cat(cor,flo), wo, bo, 1)) # [B,126,H,W]
    return cat(out, flow)                         # [B,128,H,W]

Sharding: pure data parallel, one image per NeuronCore (B=8, 8 cores).
Each core streams its image through the 5-layer pipeline in 24 4-row
blocks with all intermediate activations resident in SBUF at full image
height (no pass/halo recompute); only corr is streamed in and the
126-channel output written back.  Convs are PE matmuls with channels on
the partition dim: for each tap the shifted input window is a strided AP
into a zero-padded SBUF image, accumulated in PSUM over taps and k-tiles
(bf16 operands, fp32 PSUM — bf16 is 1 row/cycle through the PE like
fp32r but halves LDWEIGHTS + operand SBUF traffic; rel-err ~4e-3 vs the
2e-2 budget).  All M<=64 weights are padded to M=128: half-width
(col_grp=h0) LDWEIGHTS mixed with full-width ones break weight-load/
matmul overlap and cost ~110ns per matmul, while matmul time depends
only on the free dim, not M.  The final concat of `flow` into channels
126:128 happens on the host.

f2 runs in fp8(e4m3) MatmulPerfMode.DoubleRow: each DR matmul contracts
TWO taps (lhsT [K,2,M], rhs [K,2,N]) in the same 512-cycle stream,
taking f2 from 9 matmuls/block to 4 paired + 1 single (4608c -> 2560c,
~21us).  End-to-end rel-err with f2-only fp8 is ~9e-3 (sim-verified)
vs the 2e-2 budget.  The PE ifmap walker only supports the pair dim
plus ONE more (merged) dim, so the paired windows must be contiguous:
f1's output is stored as THREE 128-wide column-shifted planes
flo3[plane dw][c] = flo[c+dw-1] (f1's ACT writes the center plane, the
otherwise-idle DVE copies the +-1 shifted planes), making every 3x3-tap
window a flat 512-run and the pair AP [K, (delta,2), (1,512)].
"""

import ml_dtypes
import numpy as np

import bass_rust
import concourse.mybir as mybir
import concourse.tile as tile
from concourse import bacc
from concourse.bass_utils import run_bass_kernel_spmd

H, W = 96, 128
CIN_CORR = 324
WP = W + 2  # pad-1 padded row width (3x3 convs)
NBLK = H // 4  # 4-row output blocks streamed through the pipeline
F32 = mybir.dt.float32
BF16 = mybir.dt.bfloat16
F8 = mybir.dt.float8e4
NPBF16 = ml_dtypes.bfloat16
NPF8 = ml_dtypes.float8_e4m3fn
RELU = mybir.ActivationFunctionType.Relu
COPY = mybir.ActivationFunctionType.Copy
DR = mybir.MatmulPerfMode.DoubleRow

ZELEMS = 512  # zeros tile length (>= 2 x buffer rows; 512 for warmup rhs)
# Zero-weight warmup matmuls: bridge PE start (~8.1us) to the first
# stack-chunk DMA arrival (~11.5-12.5us) with NO gap, so the 1.2->2.4GHz
# ramp (3us continuous) completes on useless work and the real stream
# starts at full clock.  Any PE gap resets the ramp: ~3us of 427ns
# matmuls.  13 x 512-free: ~7 at 427ns (ramping) + ~6 at 213ns.
NWARM = 13

# flo3 plane geometry: [128, 3, H+4, 128]; plane stride in elements.
FLO_ROWS = H + 4
FLO_PLANE = FLO_ROWS * 128

# f2 DoubleRow tap pairing: taps (dh,dw); pair = (base_tap, partner_tap)
# with constant AP delta = (dw'-dw)*FLO_PLANE + (dh'-dh)*128.  Windows
# for tap (dh,dw) on block cc: flo3[:, dw, cc+1+dh : cc+5+dh, :].
F2_PAIRS = [((0, 0), (0, 1)), ((1, 0), (0, 2)), ((1, 1), (1, 2)), ((2, 0), (2, 1))]
F2_SINGLE = (2, 2)

# Row maps.  cor1 buffers: buffer row r holds image row r-2 (rows 1
# and H+2 are the zero-pad rows the edge taps read; 0 and H+3 unused).
# flo3 planes use the same row map at width 128 (plane dw pre-shifted
# by dw-1 columns).  catpad buffers: buffer row r holds cat row r-1
# (rows 0 and H+1 zero).


def _zero_borders(nc, buf, zrows):
    """Zero the conv-padding bytes of a padded image buffer with vector-
    engine memsets: cols {0,1} and {128,129} of every row (cols 1/128 are
    interior and overwritten by the relu writes that follow), plus the
    vertical-padding zero rows the edge taps read.  These ride the
    otherwise-idle DVE queue: as scalar-ACT copies they serialized in
    front of the first f1 relu and stalled the f2 pipeline ~2-3us."""
    for off in (0, W):
        nc.vector.memset(buf[:, :, off : off + 2], 0.0)
    for zrow in zrows:
        nc.vector.memset(buf[:, zrow : zrow + 1, :], 0.0)


def _pair_rhs(flo3, cc, base, partner):
    """rhs AP for a DoubleRow tap pair: the base tap's contiguous
    [4x128] window with an inserted (delta, 2) pair dim."""
    (dh, dw), (dh2, dw2) = base, partner
    delta = (dw2 - dw) * FLO_PLANE + (dh2 - dh) * 128
    rhs = flo3[:, dw, cc + 1 + dh : cc + 5 + dh, :].copy()
    ap = rhs.ap.to_list()  # [(pstride,128), (128,4), (1,128)]
    rhs.ap = bass_rust.VecI64Pair([ap[0], (delta, 2), (1, 512)])
    return rhs


def build_module():
    nc = bacc.Bacc(trn_type="TRN2", target_bir_lowering=False)
    # corr is zero-padded to 384 channels on the host so the three c1
    # k-tiles are a single DMA and a uniform K=128 contraction.
    corr = nc.dram_tensor("corr", [384, H, W], BF16, kind="ExternalInput").ap()
    corr_r = corr.rearrange("(kt p) h w -> p kt h w", kt=3)
    # f1 im2col K is padded 98 -> 128: partial-K LDWEIGHTS mixed with
    # full-K ones cost ~160ns per f1 matmul (same penalty class as
    # half-width col_grp loads).
    stackh = nc.dram_tensor("stackh", [128, H + 4, 128], BF16, kind="ExternalInput").ap()
    wc1p = nc.dram_tensor("wc1p", [128, 3, 256], BF16, kind="ExternalInput").ap()
    # wc2 packed WITHOUT the 192->256 M-pad: the ps1 matmul reads cols
    # 64:192 (couts 64:192, full 128-wide LDWEIGHTS) and its relu keeps
    # psum partitions 64:128 (couts 128:192) -> catpad2[64:128].  Saves
    # 0.59MB of prologue DMA, which is bandwidth-bound.
    wc2p = nc.dram_tensor("wc2p", [128, 9, 2, 192], BF16, kind="ExternalInput").ap()
    wf1p = nc.dram_tensor("wf1p", [128, 128], BF16, kind="ExternalInput").ap()
    wf2p = nc.dram_tensor("wf2p", [128, 9, 128], F8, kind="ExternalInput").ap()
    wop = nc.dram_tensor("wop", [128, 2, 9, 126], BF16, kind="ExternalInput").ap()
    biasp = nc.dram_tensor("biasp", [128, 8], F32, kind="ExternalInput").ap()
    out = nc.dram_tensor("out", [126, H, W], F32, kind="ExternalOutput").ap()

    with tile.TileContext(nc) as tc:
        with (
            tc.tile_pool(name="wpool", bufs=1) as wpool,
            tc.tile_pool(name="pspool", space="PSUM", bufs=8) as pspool,
            tc.tile_pool(name="spool", bufs=4) as spool,
            tc.tile_pool(name="opool", bufs=3) as opool,
        ):
            wc1s = wpool.tile([128, 3, 256], BF16, name="wc1s")
            wc2s = wpool.tile([128, 9, 2, 192], BF16, name="wc2s")
            wf1s = wpool.tile([128, 128], BF16, name="wf1s")
            wf2s = wpool.tile([128, 9, 128], F8, name="wf2s")
            wos = wpool.tile([128, 2, 9, 126], BF16, name="wos")
            bs = wpool.tile([128, 8], F32, name="bs")
            zsb = wpool.tile([128, ZELEMS], BF16, name="zsb")
            scr = wpool.tile([128, 1], F32, name="scr")
            # full-height intermediates, written once per row (no halo)
            stack2 = wpool.tile([128, H + 4, 128], BF16, name="stack2")
            flo3 = wpool.tile([128, 3, FLO_ROWS, 128], F8, name="flo3")
            cor1a = wpool.tile([128, H + 4, WP], BF16, name="cor1a")
            cor1b = wpool.tile([128, H + 4, WP], BF16, name="cor1b")
            catpad1 = wpool.tile([128, H + 2, WP], BF16, name="catpad1")
            catpad2 = wpool.tile([128, H + 2, WP], BF16, name="catpad2")

            # --- setup.  The zeros tile is a vector-engine memset (a DMA
            # through the Activation-queue DGE lands ~14us late and the
            # zero-border ACTs -> f1 relu -> f2 chain all wait on it).
            # The prologue (8-20us) is DMA-BANDWIDTH-bound and the 16 DMA
            # engines FAIR-SHARE across the sync/gpsimd/scalar queues: a
            # fat parallel queue halves the corr (ct) stream's rate.  So
            # ALL sizable early transfers serialize through the SYNC
            # queue in exact need order (queue position = pacing, full
            # bandwidth): stack[0:28], ct0-2, wc2s kt0, ct3, wc2s kt1.
            # gpsimd carries only the small bs/wf2s/wc1s; scalar only
            # wf1s (so the f1 relus start ~14us, not behind big
            # triggers).  wos and the remaining stack chunks ride the
            # sync queue in-loop (idx 3/4/6/9) -- not needed before
            # ~30-100us.
            nc.vector.memset(zsb, 0.0)
            nc.sync.dma_start(out=stack2[:, 0:28, :], in_=stackh[:, 0:28, :])
            # Zero-weight warmup matmuls bridge the wait for the first
            # stack DMA: the PE needs ~3us of continuous execution to
            # ramp 1.2GHz -> 2.4GHz, so by the time real work arrives
            # the clock is at full speed.
            psd = pspool.tile([128, 4, 128], F32, tag="ps", name="psdum")
            zv1 = zsb.rearrange("p (a b) -> p a b", a=1)
            for i in range(NWARM):
                nc.tensor.matmul(
                    psd, zsb[:, 0:128], zv1, start=(i == 0), stop=(i == NWARM - 1)
                )
            nc.scalar.dma_start(out=wf1s, in_=wf1p)
            nc.gpsimd.dma_start(out=bs, in_=biasp)
            nc.gpsimd.dma_start(out=wf2s, in_=wf2p)
            nc.gpsimd.dma_start(out=wc1s, in_=wc1p)
            # prewarm the Relu activation table off the critical path
            nc.scalar.activation(scr, zsb[:, 0:1], RELU)
            # flo3 zero regions: vertical pad rows 1 and H+2 (all three
            # planes), plane0 col 0 (left pad), plane2 col 127 (right
            # pad).  Interior rows/cols are written by ACT/DVE each block.
            for zrow in (1, H + 2):
                nc.vector.memset(flo3[:, :, zrow : zrow + 1, :], 0.0)
            nc.vector.memset(flo3[:, 0, :, 0:1], 0.0)
            nc.vector.memset(flo3[:, 2, :, 127:128], 0.0)
            _zero_borders(nc, cor1a, (1, H + 2))
            _zero_borders(nc, cor1b, (1, H + 2))
            _zero_borders(nc, catpad1, (0, H + 1))
            _zero_borders(nc, catpad2, (0, H + 1))

            def emit_f1(rr):
                # 7x7 conv, 2 -> 128 channels.  Input is a full host-side
                # im2col: stack2 partition cin*49+dh*7+dw holds the
                # zero-padded flow image shifted by (dh, dw), so one K=98
                # matmul computes a whole block.  The relu writes the fp8
                # center plane of flo3; the DVE then copies the +-1
                # column-shifted planes the f2 DoubleRow windows read.
                ps = pspool.tile([128, 4, 128], F32, tag="ps", name=f"psf1_{rr}")
                i = rr + 2
                nc.tensor.matmul(ps, wf1s, stack2[:, i : i + 4, :], start=True, stop=True)
                nc.scalar.activation(flo3[:, 1, i : i + 4, :], ps, RELU, bias=bs[:, 4:5])
                nc.vector.tensor_copy(
                    flo3[:, 0, i : i + 4, 1:128], flo3[:, 1, i : i + 4, 0:127]
                )
                nc.vector.tensor_copy(
                    flo3[:, 2, i : i + 4, 0:127], flo3[:, 1, i : i + 4, 1:128]
                )

            def emit_f2(cc):
                # 3x3 conv, 128 -> 64 channels -> catpad2[0:64], fp8
                # DoubleRow: 4 paired taps + 1 single.  The weights sit in
                # lhsT columns 0:64 (64:128 zero), so the conv lands
                # directly on psum partitions 0:64 and the relu writes
                # catpad2[0:64] straight from PSUM.  (catpad2 partition
                # map: 0:64 = f2 couts = cat ch 192:256, 64:128 = c2
                # couts 128:192; wop kt1 rows match.)
                ps = pspool.tile([128, 4, 128], F32, tag="ps", name=f"psf2_{cc}")
                for j, (base, partner) in enumerate(F2_PAIRS):
                    nc.tensor.matmul(
                        ps,
                        wf2s[:, 2 * j : 2 * j + 2, :],
                        _pair_rhs(flo3, cc, base, partner),
                        start=(j == 0),
                        stop=False,
                        perf_mode=DR,
                    )
                dh, dw = F2_SINGLE
                nc.tensor.matmul(
                    ps,
                    wf2s[:, 8, :],
                    flo3[:, dw, cc + 1 + dh : cc + 5 + dh, :],
                    start=False,
                    stop=True,
                )
                nc.scalar.activation(
                    catpad2[0:64, cc + 1 : cc + 5, 1 : 1 + W],
                    ps[0:64],
                    RELU,
                    bias=bs[0:64, 5:6],
                )

            def emit_c1_dma(rr):
                ct = spool.tile([128, 3, 4, 128], BF16, tag="corr", name=f"ct_{rr}")
                nc.sync.dma_start(out=ct, in_=corr_r[:, :, rr : rr + 4, :])
                return ct

            def emit_c1_mm(rr, ct):
                ps0 = pspool.tile([128, 4, 128], F32, tag="ps", name=f"psc1a_{rr}")
                ps1 = pspool.tile([128, 4, 128], F32, tag="ps", name=f"psc1b_{rr}")
                for kt in range(3):
                    nc.tensor.matmul(
                        ps0, wc1s[:, kt, 0:128], ct[:, kt], start=(kt == 0), stop=(kt == 2)
                    )
                    nc.tensor.matmul(
                        ps1, wc1s[:, kt, 128:256], ct[:, kt], start=(kt == 0), stop=(kt == 2)
                    )
                r = rr + 2
                nc.scalar.activation(cor1a[:, r : r + 4, 1 : 1 + W], ps0, RELU, bias=bs[:, 0:1])
                nc.scalar.activation(cor1b[:, r : r + 4, 1 : 1 + W], ps1, RELU, bias=bs[:, 1:2])

            def emit_c2(cc):
                # ps1 reads lhsT cols 64:192 (couts 64:192, full-width
                # LDWEIGHTS, no M-pad DMA) and keeps psum partitions
                # 64:128 = couts 128:192 -> catpad2[64:128].
                ps0 = pspool.tile([128, 4, 128], F32, tag="ps", name=f"psc2a_{cc}")
                ps1 = pspool.tile([128, 4, 128], F32, tag="ps", name=f"psc2b_{cc}")
                k = 0
                for kt, src_ in enumerate((cor1a, cor1b)):
                    for dh in range(3):
                        for dw in range(3):
                            i = cc + 1 + dh
                            rhs = src_[:, i : i + 4, dw : dw + 128]
                            tap = 3 * dh + dw
                            nc.tensor.matmul(
                                ps0, wc2s[:, tap, kt, 0:128], rhs, start=(k == 0), stop=(k == 17)
                            )
                            nc.tensor.matmul(
                                ps1,
                                wc2s[:, tap, kt, 64:192],
                                rhs,
                                start=(k == 0),
                                stop=(k == 17),
                            )
                            k += 1
                r = cc + 1
                nc.scalar.activation(catpad1[:, r : r + 4, 1 : 1 + W], ps0, RELU, bias=bs[:, 2:3])
                nc.scalar.activation(
                    catpad2[64:128, r : r + 4, 1 : 1 + W],
                    ps1[64:128],
                    RELU,
                    bias=bs[64:128, 3:4],
                )

            def emit_o(oo, split=False):
                ps = pspool.tile([128, 4, 128], F32, tag="ps", name=f"pso_{oo}")
                k = 0
                for kt, src_ in enumerate((catpad1, catpad2)):
                    for dh in range(3):
                        for dw in range(3):
                            i = oo + dh
                            nc.tensor.matmul(
                                ps[0:126],
                                wos[:, kt, 3 * dh + dw, :],
                                src_[:, i : i + 4, dw : dw + 128],
                                start=(k == 0),
                                stop=(k == 17),
                            )
                            k += 1
                ob = opool.tile([128, 4, 128], F32, tag="ob", name=f"ob_{oo}")
                if split:
                    # last block: 2-row ACT halves so the final out DMA
                    # starts ~0.3us earlier (it ends the kernel).
                    for h in range(2):
                        nc.scalar.activation(
                            ob[0:126, 2 * h : 2 * h + 2],
                            ps[0:126, 2 * h : 2 * h + 2],
                            RELU,
                            bias=bs[0:126, 6:7],
                        )
                        nc.sync.dma_start(
                            out=out[:, oo + 2 * h : oo + 2 * h + 2, :],
                            in_=ob[0:126, 2 * h : 2 * h + 2],
                        )
                else:
                    nc.scalar.activation(ob[0:126], ps[0:126], RELU, bias=bs[0:126, 6:7])
                    nc.sync.dma_start(out=out[:, oo : oo + 4, :], in_=ob[0:126])

            # --- the streamed pipeline.  c1 has only 6 matmuls per block
            # against ~2us of corr DMA, so on its own it starves the PE;
            # staggering f2 (5 matmuls), c2 (36) and o (18) behind it
            # keeps the PE dense while corr streams.
            # Prologue ordering principle: the Tensor queue is in-order,
            # so ACT/DVE-gated work (f2 reads flo3 = f1 relu + DVE plane
            # copies, two hops deep) must sit BEHIND DMA-fed work (f1
            # from stack2, c1 from the ct stream) — interleaving c1
            # between the early f1/f2 blocks keeps the PE dense while
            # the scalar/vector queues drain their startup backlog.
            cts_q = {}
            for j in range(3):
                cts_q[j] = emit_c1_dma(4 * j)
            # wc2s k-halves interleave the ct stream on sync: c2(0) only
            # needs kt=0 for its first 18 matmuls (~22us), kt=1 by ~25us
            nc.sync.dma_start(out=wc2s[:, :, 0, :], in_=wc2p[:, :, 0, :])
            cts_q[3] = emit_c1_dma(12)
            nc.sync.dma_start(out=wc2s[:, :, 1, :], in_=wc2p[:, :, 1, :])
            for j in range(4):
                emit_f1(4 * j)
            emit_c1_mm(0, cts_q.pop(0))
            emit_f1(16)
            emit_f1(20)
            emit_c1_mm(4, cts_q.pop(1))
            emit_c1_mm(8, cts_q.pop(2))
            emit_f2(0)
            emit_f2(4)
            emit_c2(0)
            # o trails c2 by 4 blocks in steady state (so it never waits
            # on the same-iteration c2 drain), but the tail is compressed:
            # o(22) rides with o(21) one iteration early, trading a ~0.6us
            # ACT wait for a whole 3.9us single-stream drain iteration.
            for idx in range(3, NBLK + 3):
                if idx + 1 < NBLK:
                    cts_q[idx + 1] = emit_c1_dma(4 * (idx + 1))
                # remaining stack chunks + wos ride the (now slack) sync
                # queue well ahead of their first consumer
                if idx == 3:
                    nc.sync.dma_start(out=stack2[:, 28:52, :], in_=stackh[:, 28:52, :])
                elif idx == 4:
                    nc.sync.dma_start(out=wos, in_=wop)
                elif idx == 6:
                    nc.sync.dma_start(out=stack2[:, 52:76, :], in_=stackh[:, 52:76, :])
                elif idx == 9:
                    nc.sync.dma_start(out=stack2[:, 76:100, :], in_=stackh[:, 76:100, :])
                if idx + 3 < NBLK:
                    emit_f1(4 * (idx + 3))
                if idx < NBLK:
                    emit_c1_mm(4 * idx, cts_q.pop(idx))
                if idx - 1 < NBLK:
                    emit_f2(4 * (idx - 1))
                if idx - 2 < NBLK:
                    emit_c2(4 * (idx - 2))
                if 0 <= idx - 4 < NBLK - 2:
                    emit_o(4 * (idx - 4))
                if idx == NBLK + 1:
                    emit_o(4 * (NBLK - 2))
                if idx == NBLK + 2:
                    emit_o(4 * (NBLK - 1), split=True)
    nc.compile()
    return nc


def pack_params(wc1, bc1, wc2, bc2, wf1, bf1, wf2, bf2, wo, bo):
    """Host-side repack of OIHW conv weights into the lhsT layouts the
    kernel's matmuls read ([K partitions, ..., M])."""
    f = np.float32
    wc1p = np.zeros((128, 3, 256), f)
    w = wc1[:, :, 0, 0]  # [256, 324]
    for kt in range(3):
        kk = min(128, CIN_CORR - kt * 128)
        wc1p[0:kk, kt, :] = w[:, kt * 128 : kt * 128 + kk].T
    wc2p = np.zeros((128, 9, 2, 192), f)
    for dh in range(3):
        for dw in range(3):
            for kt in range(2):
                wc2p[:, 3 * dh + dw, kt, :] = wc2[:, kt * 128 : kt * 128 + 128, dh, dw].T
    wf1p = np.zeros((128, 128), f)
    for cin in range(2):
        for dh in range(7):
            for dw in range(7):
                wf1p[cin * 49 + dh * 7 + dw, :] = wf1[:, cin, dh, dw]
    # f2 weights in fp8, DoubleRow pair order: slots 2j/2j+1 hold the
    # base/partner taps of F2_PAIRS[j], slot 8 the single tap.  couts sit
    # in lhsT columns 0:64 (psum partition targeting -> catpad2[0:64]).
    wf2p = np.zeros((128, 9, 128), f)
    for j, (base, partner) in enumerate(F2_PAIRS):
        for s, (dh, dw) in enumerate((base, partner)):
            wf2p[:, 2 * j + s, 0:64] = wf2[:, :, dh, dw].T
    dh, dw = F2_SINGLE
    wf2p[:, 8, 0:64] = wf2[:, :, dh, dw].T
    # o kt1 rows follow the catpad2 partition map: 0:64 = f2 couts
    # (cat ch 192:256), 64:128 = c2 couts 128:192.
    wop = np.zeros((128, 2, 9, 126), f)
    for dh in range(3):
        for dw in range(3):
            tap = 3 * dh + dw
            wop[:, 0, tap, :] = wo[:, 0:128, dh, dw].T
            wop[0:64, 1, tap, :] = wo[:, 192:256, dh, dw].T
            wop[64:128, 1, tap, :] = wo[:, 128:192, dh, dw].T
    biasp = np.zeros((128, 8), f)
    biasp[:, 0] = bc1[0:128]
    biasp[:, 1] = bc1[128:256]
    biasp[:, 2] = bc2[0:128]
    biasp[64:128, 3] = bc2[128:192]
    biasp[:, 4] = bf1
    biasp[0:64, 5] = bf2
    biasp[0:126, 6] = bo
    return {
        "wc1p": wc1p.astype(NPBF16),
        "wc2p": wc2p.astype(NPBF16),
        "wf1p": wf1p.astype(NPBF16),
        "wf2p": wf2p.astype(NPF8),
        "wop": wop.astype(NPBF16),
        "biasp": biasp,
    }


def build_stackh(flow_b):
    """Full f1 im2col: [98, H+4, 128], partition cin*49+dh*7+dw holds the
    zero-padded (pad 3) flow image shifted by (dh, dw); row i <-> f1 output
    row i-2."""
    fz = np.zeros((2, H + 10, W + 6), NPBF16)
    fz[:, 5 : 5 + H, 3 : 3 + W] = flow_b
    s = np.zeros((128, H + 4, 128), NPBF16)
    for cin in range(2):
        for dh in range(7):
            for dw in range(7):
                # output row r (= buffer row r+2) col c reads fz row r+dh+2, col c+dw
                s[cin * 49 + dh * 7 + dw] = fz[cin, dh : dh + H + 4, dw : dw + 128]
    return s


_MODULE = None


def _get_module():
    global _MODULE
    if _MODULE is None:
        _MODULE = build_module()
    return _MODULE


def make_in_maps(**inputs):
    a = {
        k: np.ascontiguousarray(np.asarray(v), dtype=np.float32)
        for k, v in inputs.items()
    }
    packed = pack_params(
        a["wc1"], a["bc1"], a["wc2"], a["bc2"], a["wf1"], a["bf1"],
        a["wf2"], a["bf2"], a["wo"], a["bo"],
    )
    corr_bf = a["corr"].astype(NPBF16)
    in_maps = []
    for b in range(8):
        m = dict(packed)
        cp = np.zeros((384, H, W), NPBF16)
        cp[0:CIN_CORR] = corr_bf[b]
        m["corr"] = cp
        m["stackh"] = build_stackh(a["flow"][b])
        in_maps.append(m)
    return in_maps, a["flow"]


def assemble_output(results, flow):
    out = np.empty((8, 128, H, W), np.float32)
    for b in range(8):
        out[b, :126] = results[b]["out"]
        out[b, 126:] = flow[b]
    return out


def run(trace=False, **inputs):
    in_maps, flow = make_in_maps(**inputs)
    nc = _get_module()
    res = run_bass_kernel_spmd(nc, in_maps, core_ids=list(range(8)), trace=trace)
    return assemble_output(res.results, flow), res


def kernel(**inputs):
    out, _ = run(trace=False, **inputs)
    return out


# revision 23
# speedup vs baseline: 1.0318x; 1.0024x over previous
"""Trainium2 Bass kernel for BasicMotionEncoder (RAFT motion encoder).

Network (all stride-1 convs, NCHW, fp32 in/out):
    cor  = relu(conv1x1(corr, wc1, bc1))          # [B,256,H,W]
    cor  = relu(conv3x3(cor,  wc2, bc2, pad 1))   # [B,192,H,W]
    flo  = relu(conv7x7(flow, wf1, bf1, pad 3))   # [B,128,H,W]
    flo  = relu(conv3x3(flo,  wf2, bf2, pad 1))   # [B,64,H,W]
    out  = relu(conv3x3(cat(cor,flo), wo, bo, 1)) # [B,126,H,W]
    return cat(out, flow)                         # [B,128,H,W]

Sharding: pure data parallel, one image per NeuronCore (B=8, 8 cores).
Each core streams its image through the 5-layer pipeline in 24 4-row
blocks with all intermediate activations resident in SBUF at full image
height (no pass/halo recompute); only corr is streamed in and the
126-channel output written back.  Convs are PE matmuls with channels on
the partition dim: for each tap the shifted input window is a strided AP
into a zero-padded SBUF image, accumulated in PSUM over taps and k-tiles
(bf16 operands, fp32 PSUM — bf16 is 1 row/cycle through the PE like
fp32r but halves LDWEIGHTS + operand SBUF traffic; rel-err ~4e-3 vs the
2e-2 budget).  All M<=64 weights are padded to M=128: half-width
(col_grp=h0) LDWEIGHTS mixed with full-width ones break weight-load/
matmul overlap and cost ~110ns per matmul, while matmul time depends
only on the free dim, not M.  The final concat of `flow` into channels
126:128 happens on the host.

f2 runs in fp8(e4m3) MatmulPerfMode.DoubleRow: each DR matmul contracts
TWO taps (lhsT [K,2,M], rhs [K,2,N]) in the same 512-cycle stream,
taking f2 from 9 matmuls/block to 4 paired + 1 single (4608c -> 2560c,
~21us).  End-to-end rel-err with f2-only fp8 is ~9e-3 (sim-verified)
vs the 2e-2 budget.  The PE ifmap walker only supports the pair dim
plus ONE more (merged) dim, so the paired windows must be contiguous:
f1's output is stored as THREE 128-wide column-shifted planes
flo3[plane dw][c] = flo[c+dw-1] (f1's ACT writes the center plane, the
otherwise-idle DVE copies the +-1 shifted planes), making every 3x3-tap
window a flat 512-run and the pair AP [K, (delta,2), (1,512)].
"""

import ml_dtypes
import numpy as np

import bass_rust
import concourse.mybir as mybir
import concourse.tile as tile
from concourse import bacc
from concourse.bass_utils import run_bass_kernel_spmd

H, W = 96, 128
CIN_CORR = 324
WP = W + 2  # pad-1 padded row width (3x3 convs)
NBLK = H // 4  # 4-row output blocks streamed through the pipeline
F32 = mybir.dt.float32
BF16 = mybir.dt.bfloat16
F8 = mybir.dt.float8e4
NPBF16 = ml_dtypes.bfloat16
NPF8 = ml_dtypes.float8_e4m3fn
RELU = mybir.ActivationFunctionType.Relu
COPY = mybir.ActivationFunctionType.Copy
DR = mybir.MatmulPerfMode.DoubleRow

ZELEMS = 512  # zeros tile length (>= 2 x buffer rows; 512 for warmup rhs)
# Zero-weight warmup matmuls: bridge PE start (~8.1us) to the first
# stack-chunk DMA arrival (~11.5-12.5us) with NO gap, so the 1.2->2.4GHz
# ramp (3us continuous) completes on useless work and the real stream
# starts at full clock.  Any PE gap resets the ramp: ~3us of 427ns
# matmuls.  11 x 512-free: ~7 at 427ns (ramping) + ~4 at 213ns.
NWARM = 11

# flo3 plane geometry: [128, 3, H+4, 128]; plane stride in elements.
FLO_ROWS = H + 4
FLO_PLANE = FLO_ROWS * 128

# f2 DoubleRow tap pairing: taps (dh,dw); pair = (base_tap, partner_tap)
# with constant AP delta = (dw'-dw)*FLO_PLANE + (dh'-dh)*128.  Windows
# for tap (dh,dw) on block cc: flo3[:, dw, cc+1+dh : cc+5+dh, :].
F2_PAIRS = [((0, 0), (0, 1)), ((1, 0), (0, 2)), ((1, 1), (1, 2)), ((2, 0), (2, 1))]
F2_SINGLE = (2, 2)

# Row maps.  cor1 buffers: buffer row r holds image row r-2 (rows 1
# and H+2 are the zero-pad rows the edge taps read; 0 and H+3 unused).
# flo3 planes use the same row map at width 128 (plane dw pre-shifted
# by dw-1 columns).  catpad buffers: buffer row r holds cat row r-1
# (rows 0 and H+1 zero).


def _zero_borders(nc, buf, zrows):
    """Zero the conv-padding bytes of a padded image buffer with vector-
    engine memsets: cols {0,1} and {128,129} of every row (cols 1/128 are
    interior and overwritten by the relu writes that follow), plus the
    vertical-padding zero rows the edge taps read.  These ride the
    otherwise-idle DVE queue: as scalar-ACT copies they serialized in
    front of the first f1 relu and stalled the f2 pipeline ~2-3us."""
    for off in (0, W):
        nc.vector.memset(buf[:, :, off : off + 2], 0.0)
    for zrow in zrows:
        nc.vector.memset(buf[:, zrow : zrow + 1, :], 0.0)


def _pair_rhs(flo3, cc, base, partner):
    """rhs AP for a DoubleRow tap pair: the base tap's contiguous
    [4x128] window with an inserted (delta, 2) pair dim."""
    (dh, dw), (dh2, dw2) = base, partner
    delta = (dw2 - dw) * FLO_PLANE + (dh2 - dh) * 128
    rhs = flo3[:, dw, cc + 1 + dh : cc + 5 + dh, :].copy()
    ap = rhs.ap.to_list()  # [(pstride,128), (128,4), (1,128)]
    rhs.ap = bass_rust.VecI64Pair([ap[0], (delta, 2), (1, 512)])
    return rhs


def build_module():
    nc = bacc.Bacc(trn_type="TRN2", target_bir_lowering=False)
    # corr is zero-padded to 384 channels on the host so the three c1
    # k-tiles are a single DMA and a uniform K=128 contraction.
    corr = nc.dram_tensor("corr", [384, H, W], BF16, kind="ExternalInput").ap()
    corr_r = corr.rearrange("(kt p) h w -> p kt h w", kt=3)
    # f1 im2col K is padded 98 -> 128: partial-K LDWEIGHTS mixed with
    # full-K ones cost ~160ns per f1 matmul (same penalty class as
    # half-width col_grp loads).
    stackh = nc.dram_tensor("stackh", [128, H + 4, 128], BF16, kind="ExternalInput").ap()
    wc1p = nc.dram_tensor("wc1p", [128, 3, 256], BF16, kind="ExternalInput").ap()
    # wc2 packed WITHOUT the 192->256 M-pad: the ps1 matmul reads cols
    # 64:192 (couts 64:192, full 128-wide LDWEIGHTS) and its relu keeps
    # psum partitions 64:128 (couts 128:192) -> catpad2[64:128].  Saves
    # 0.59MB of prologue DMA, which is bandwidth-bound.
    wc2p = nc.dram_tensor("wc2p", [128, 9, 2, 192], BF16, kind="ExternalInput").ap()
    wf1p = nc.dram_tensor("wf1p", [128, 128], BF16, kind="ExternalInput").ap()
    wf2p = nc.dram_tensor("wf2p", [128, 9, 128], F8, kind="ExternalInput").ap()
    wop = nc.dram_tensor("wop", [128, 2, 9, 126], BF16, kind="ExternalInput").ap()
    biasp = nc.dram_tensor("biasp", [128, 8], F32, kind="ExternalInput").ap()
    out = nc.dram_tensor("out", [126, H, W], F32, kind="ExternalOutput").ap()

    with tile.TileContext(nc) as tc:
        with (
            tc.tile_pool(name="wpool", bufs=1) as wpool,
            tc.tile_pool(name="pspool", space="PSUM", bufs=8) as pspool,
            tc.tile_pool(name="spool", bufs=4) as spool,
            tc.tile_pool(name="opool", bufs=3) as opool,
        ):
            wc1s = wpool.tile([128, 3, 256], BF16, name="wc1s")
            wc2s = wpool.tile([128, 9, 2, 192], BF16, name="wc2s")
            wf1s = wpool.tile([128, 128], BF16, name="wf1s")
            wf2s = wpool.tile([128, 9, 128], F8, name="wf2s")
            wos = wpool.tile([128, 2, 9, 126], BF16, name="wos")
            bs = wpool.tile([128, 8], F32, name="bs")
            zsb = wpool.tile([128, ZELEMS], BF16, name="zsb")
            scr = wpool.tile([128, 1], F32, name="scr")
            # full-height intermediates, written once per row (no halo)
            stack2 = wpool.tile([128, H + 4, 128], BF16, name="stack2")
            flo3 = wpool.tile([128, 3, FLO_ROWS, 128], F8, name="flo3")
            cor1a = wpool.tile([128, H + 4, WP], BF16, name="cor1a")
            cor1b = wpool.tile([128, H + 4, WP], BF16, name="cor1b")
            catpad1 = wpool.tile([128, H + 2, WP], BF16, name="catpad1")
            catpad2 = wpool.tile([128, H + 2, WP], BF16, name="catpad2")

            # --- setup.  The zeros tile is a vector-engine memset (a DMA
            # through the Activation-queue DGE lands ~14us late and the
            # zero-border ACTs -> f1 relu -> f2 chain all wait on it).
            # The prologue (8-20us) is DMA-BANDWIDTH-bound and the 16 DMA
            # engines FAIR-SHARE across the sync/gpsimd/scalar queues: a
            # fat parallel queue halves the corr (ct) stream's rate.  So
            # ALL sizable early transfers serialize through the SYNC
            # queue in exact need order (queue position = pacing, full
            # bandwidth): stack[0:28], ct0-2, wc2s kt0, ct3, wc2s kt1.
            # gpsimd carries only the small bs/wf2s/wc1s; scalar only
            # wf1s (so the f1 relus start ~14us, not behind big
            # triggers).  wos and the remaining stack chunks ride the
            # sync queue in-loop (idx 3/4/6/9) -- not needed before
            # ~30-100us.
            nc.vector.memset(zsb, 0.0)
            nc.sync.dma_start(out=stack2[:, 0:28, :], in_=stackh[:, 0:28, :])
            # Zero-weight warmup matmuls bridge the wait for the first
            # stack DMA: the PE needs ~3us of continuous execution to
            # ramp 1.2GHz -> 2.4GHz, so by the time real work arrives
            # the clock is at full speed.
            psd = pspool.tile([128, 4, 128], F32, tag="ps", name="psdum")
            zv1 = zsb.rearrange("p (a b) -> p a b", a=1)
            for i in range(NWARM):
                nc.tensor.matmul(
                    psd, zsb[:, 0:128], zv1, start=(i == 0), stop=(i == NWARM - 1)
                )
            # wf1s/wc1s on SYNC too: the gpsimd/scalar DGEs only boot at
            # ~11.4us and crawl (~75GB/s) -- wc1s on gpsimd landed 16.1us
            # and gated c1(0) by 3.4us.  gpsimd keeps only bs/wf2s (tiny,
            # needed at ~13.2/17.5us).
            nc.sync.dma_start(out=wf1s, in_=wf1p)
            nc.sync.dma_start(out=wc1s, in_=wc1p)
            nc.gpsimd.dma_start(out=bs, in_=biasp)
            nc.gpsimd.dma_start(out=wf2s, in_=wf2p)
            # prewarm the Relu activation table off the critical path
            nc.scalar.activation(scr, zsb[:, 0:1], RELU)
            # flo3 zero regions: vertical pad rows 1 and H+2 (all three
            # planes), plane0 col 0 (left pad), plane2 col 127 (right
            # pad).  Interior rows/cols are written by ACT/DVE each block.
            for zrow in (1, H + 2):
                nc.vector.memset(flo3[:, :, zrow : zrow + 1, :], 0.0)
            nc.vector.memset(flo3[:, 0, :, 0:1], 0.0)
            nc.vector.memset(flo3[:, 2, :, 127:128], 0.0)
            _zero_borders(nc, cor1a, (1, H + 2))
            _zero_borders(nc, cor1b, (1, H + 2))
            _zero_borders(nc, catpad1, (0, H + 1))
            _zero_borders(nc, catpad2, (0, H + 1))

            def emit_f1(rr):
                # 7x7 conv, 2 -> 128 channels.  Input is a full host-side
                # im2col: stack2 partition cin*49+dh*7+dw holds the
                # zero-padded flow image shifted by (dh, dw), so one K=98
                # matmul computes a whole block.  The relu writes the fp8
                # center plane of flo3; the DVE then copies the +-1
                # column-shifted planes the f2 DoubleRow windows read.
                ps = pspool.tile([128, 4, 128], F32, tag="ps", name=f"psf1_{rr}")
                i = rr + 2
                nc.tensor.matmul(ps, wf1s, stack2[:, i : i + 4, :], start=True, stop=True)
                nc.scalar.activation(flo3[:, 1, i : i + 4, :], ps, RELU, bias=bs[:, 4:5])
                nc.vector.tensor_copy(
                    flo3[:, 0, i : i + 4, 1:128], flo3[:, 1, i : i + 4, 0:127]
                )
                nc.vector.tensor_copy(
                    flo3[:, 2, i : i + 4, 0:127], flo3[:, 1, i : i + 4, 1:128]
                )

            def emit_f2(cc):
                # 3x3 conv, 128 -> 64 channels -> catpad2[0:64], fp8
                # DoubleRow: 4 paired taps + 1 single.  The weights sit in
                # lhsT columns 0:64 (64:128 zero), so the conv lands
                # directly on psum partitions 0:64 and the relu writes
                # catpad2[0:64] straight from PSUM.  (catpad2 partition
                # map: 0:64 = f2 couts = cat ch 192:256, 64:128 = c2
                # couts 128:192; wop kt1 rows match.)
                ps = pspool.tile([128, 4, 128], F32, tag="ps", name=f"psf2_{cc}")
                for j, (base, partner) in enumerate(F2_PAIRS):
                    nc.tensor.matmul(
                        ps,
                        wf2s[:, 2 * j : 2 * j + 2, :],
                        _pair_rhs(flo3, cc, base, partner),
                        start=(j == 0),
                        stop=False,
                        perf_mode=DR,
                    )
                dh, dw = F2_SINGLE
                nc.tensor.matmul(
                    ps,
                    wf2s[:, 8, :],
                    flo3[:, dw, cc + 1 + dh : cc + 5 + dh, :],
                    start=False,
                    stop=True,
                )
                nc.scalar.activation(
                    catpad2[0:64, cc + 1 : cc + 5, 1 : 1 + W],
                    ps[0:64],
                    RELU,
                    bias=bs[0:64, 5:6],
                )

            def emit_c1_dma(rr):
                ct = spool.tile([128, 3, 4, 128], BF16, tag="corr", name=f"ct_{rr}")
                nc.sync.dma_start(out=ct, in_=corr_r[:, :, rr : rr + 4, :])
                return ct

            def emit_c1_mm(rr, ct):
                ps0 = pspool.tile([128, 4, 128], F32, tag="ps", name=f"psc1a_{rr}")
                ps1 = pspool.tile([128, 4, 128], F32, tag="ps", name=f"psc1b_{rr}")
                for kt in range(3):
                    nc.tensor.matmul(
                        ps0, wc1s[:, kt, 0:128], ct[:, kt], start=(kt == 0), stop=(kt == 2)
                    )
                    nc.tensor.matmul(
                        ps1, wc1s[:, kt, 128:256], ct[:, kt], start=(kt == 0), stop=(kt == 2)
                    )
                r = rr + 2
                nc.scalar.activation(cor1a[:, r : r + 4, 1 : 1 + W], ps0, RELU, bias=bs[:, 0:1])
                nc.scalar.activation(cor1b[:, r : r + 4, 1 : 1 + W], ps1, RELU, bias=bs[:, 1:2])

            def emit_c2(cc):
                # ps1 reads lhsT cols 64:192 (couts 64:192, full-width
                # LDWEIGHTS, no M-pad DMA) and keeps psum partitions
                # 64:128 = couts 128:192 -> catpad2[64:128].
                ps0 = pspool.tile([128, 4, 128], F32, tag="ps", name=f"psc2a_{cc}")
                ps1 = pspool.tile([128, 4, 128], F32, tag="ps", name=f"psc2b_{cc}")
                k = 0
                for kt, src_ in enumerate((cor1a, cor1b)):
                    for dh in range(3):
                        for dw in range(3):
                            i = cc + 1 + dh
                            rhs = src_[:, i : i + 4, dw : dw + 128]
                            tap = 3 * dh + dw
                            nc.tensor.matmul(
                                ps0, wc2s[:, tap, kt, 0:128], rhs, start=(k == 0), stop=(k == 17)
                            )
                            nc.tensor.matmul(
                                ps1,
                                wc2s[:, tap, kt, 64:192],
                                rhs,
                                start=(k == 0),
                                stop=(k == 17),
                            )
                            k += 1
                r = cc + 1
                nc.scalar.activation(catpad1[:, r : r + 4, 1 : 1 + W], ps0, RELU, bias=bs[:, 2:3])
                nc.scalar.activation(
                    catpad2[64:128, r : r + 4, 1 : 1 + W],
                    ps1[64:128],
                    RELU,
                    bias=bs[64:128, 3:4],
                )

            def emit_o(oo, split=False):
                ps = pspool.tile([128, 4, 128], F32, tag="ps", name=f"pso_{oo}")
                k = 0
                for kt, src_ in enumerate((catpad1, catpad2)):
                    for dh in range(3):
                        for dw in range(3):
                            i = oo + dh
                            nc.tensor.matmul(
                                ps[0:126],
                                wos[:, kt, 3 * dh + dw, :],
                                src_[:, i : i + 4, dw : dw + 128],
                                start=(k == 0),
                                stop=(k == 17),
                            )
                            k += 1
                ob = opool.tile([128, 4, 128], F32, tag="ob", name=f"ob_{oo}")
                if split:
                    # last block: 2-row ACT halves so the final out DMA
                    # starts ~0.3us earlier (it ends the kernel).
                    for h in range(2):
                        nc.scalar.activation(
                            ob[0:126, 2 * h : 2 * h + 2],
                            ps[0:126, 2 * h : 2 * h + 2],
                            RELU,
                            bias=bs[0:126, 6:7],
                        )
                        nc.sync.dma_start(
                            out=out[:, oo + 2 * h : oo + 2 * h + 2, :],
                            in_=ob[0:126, 2 * h : 2 * h + 2],
                        )
                else:
                    nc.scalar.activation(ob[0:126], ps[0:126], RELU, bias=bs[0:126, 6:7])
                    nc.sync.dma_start(out=out[:, oo : oo + 4, :], in_=ob[0:126])

            # --- the streamed pipeline.  c1 has only 6 matmuls per block
            # against ~2us of corr DMA, so on its own it starves the PE;
            # staggering f2 (5 matmuls), c2 (36) and o (18) behind it
            # keeps the PE dense while corr streams.
            # Prologue ordering principle: the Tensor queue is in-order,
            # so ACT/DVE-gated work (f2 reads flo3 = f1 relu + DVE plane
            # copies, two hops deep) must sit BEHIND DMA-fed work (f1
            # from stack2, c1 from the ct stream) — interleaving c1
            # between the early f1/f2 blocks keeps the PE dense while
            # the scalar/vector queues drain their startup backlog.
            cts_q = {}
            for j in range(3):
                cts_q[j] = emit_c1_dma(4 * j)
            # wc2s k-halves interleave the ct stream on sync: c2(0) only
            # needs kt=0 for its first 18 matmuls (~22us), kt=1 by ~25us
            nc.sync.dma_start(out=wc2s[:, :, 0, :], in_=wc2p[:, :, 0, :])
            cts_q[3] = emit_c1_dma(12)
            nc.sync.dma_start(out=wc2s[:, :, 1, :], in_=wc2p[:, :, 1, :])
            for j in range(6):
                emit_f1(4 * j)
            emit_c1_mm(0, cts_q.pop(0))
            emit_c1_mm(4, cts_q.pop(1))
            emit_c1_mm(8, cts_q.pop(2))
            emit_f2(0)
            emit_f2(4)
            emit_c2(0)
            # o trails c2 by 4 blocks in steady state (so it never waits
            # on the same-iteration c2 drain), but the tail is compressed:
            # o(22) rides with o(21) one iteration early, trading a ~0.6us
            # ACT wait for a whole 3.9us single-stream drain iteration.
            for idx in range(3, NBLK + 3):
                if idx + 1 < NBLK:
                    cts_q[idx + 1] = emit_c1_dma(4 * (idx + 1))
                # remaining stack chunks + wos ride the (now slack) sync
                # queue well ahead of their first consumer
                if idx == 3:
                    nc.sync.dma_start(out=stack2[:, 28:52, :], in_=stackh[:, 28:52, :])
                elif idx == 4:
                    nc.sync.dma_start(out=wos, in_=wop)
                elif idx == 6:
                    nc.sync.dma_start(out=stack2[:, 52:76, :], in_=stackh[:, 52:76, :])
                elif idx == 9:
                    nc.sync.dma_start(out=stack2[:, 76:100, :], in_=stackh[:, 76:100, :])
                if idx + 3 < NBLK:
                    emit_f1(4 * (idx + 3))
                if idx < NBLK:
                    emit_c1_mm(4 * idx, cts_q.pop(idx))
                if idx - 1 < NBLK:
                    emit_f2(4 * (idx - 1))
                if idx - 2 < NBLK:
                    emit_c2(4 * (idx - 2))
                if 0 <= idx - 4 < NBLK - 2:
                    emit_o(4 * (idx - 4))
                if idx == NBLK + 1:
                    emit_o(4 * (NBLK - 2))
                if idx == NBLK + 2:
                    emit_o(4 * (NBLK - 1), split=True)
    nc.compile()
    return nc


def pack_params(wc1, bc1, wc2, bc2, wf1, bf1, wf2, bf2, wo, bo):
    """Host-side repack of OIHW conv weights into the lhsT layouts the
    kernel's matmuls read ([K partitions, ..., M])."""
    f = np.float32
    wc1p = np.zeros((128, 3, 256), f)
    w = wc1[:, :, 0, 0]  # [256, 324]
    for kt in range(3):
        kk = min(128, CIN_CORR - kt * 128)
        wc1p[0:kk, kt, :] = w[:, kt * 128 : kt * 128 + kk].T
    wc2p = np.zeros((128, 9, 2, 192), f)
    for dh in range(3):
        for dw in range(3):
            for kt in range(2):
                wc2p[:, 3 * dh + dw, kt, :] = wc2[:, kt * 128 : kt * 128 + 128, dh, dw].T
    wf1p = np.zeros((128, 128), f)
    for cin in range(2):
        for dh in range(7):
            for dw in range(7):
                wf1p[cin * 49 + dh * 7 + dw, :] = wf1[:, cin, dh, dw]
    # f2 weights in fp8, DoubleRow pair order: slots 2j/2j+1 hold the
    # base/partner taps of F2_PAIRS[j], slot 8 the single tap.  couts sit
    # in lhsT columns 0:64 (psum partition targeting -> catpad2[0:64]).
    wf2p = np.zeros((128, 9, 128), f)
    for j, (base, partner) in enumerate(F2_PAIRS):
        for s, (dh, dw) in enumerate((base, partner)):
            wf2p[:, 2 * j + s, 0:64] = wf2[:, :, dh, dw].T
    dh, dw = F2_SINGLE
    wf2p[:, 8, 0:64] = wf2[:, :, dh, dw].T
    # o kt1 rows follow the catpad2 partition map: 0:64 = f2 couts
    # (cat ch 192:256), 64:128 = c2 couts 128:192.
    wop = np.zeros((128, 2, 9, 126), f)
    for dh in range(3):
        for dw in range(3):
            tap = 3 * dh + dw
            wop[:, 0, tap, :] = wo[:, 0:128, dh, dw].T
            wop[0:64, 1, tap, :] = wo[:, 192:256, dh, dw].T
            wop[64:128, 1, tap, :] = wo[:, 128:192, dh, dw].T
    biasp = np.zeros((128, 8), f)
    biasp[:, 0] = bc1[0:128]
    biasp[:, 1] = bc1[128:256]
    biasp[:, 2] = bc2[0:128]
    biasp[64:128, 3] = bc2[128:192]
    biasp[:, 4] = bf1
    biasp[0:64, 5] = bf2
    biasp[0:126, 6] = bo
    return {
        "wc1p": wc1p.astype(NPBF16),
        "wc2p": wc2p.astype(NPBF16),
        "wf1p": wf1p.astype(NPBF16),
        "wf2p": wf2p.astype(NPF8),
        "wop": wop.astype(NPBF16),
        "biasp": biasp,
    }


def build_stackh(flow_b):
    """Full f1 im2col: [98, H+4, 128], partition cin*49+dh*7+dw holds the
    zero-padded (pad 3) flow image shifted by (dh, dw); row i <-> f1 output
    row i-2."""
    fz = np.zeros((2, H + 10, W + 6), NPBF16)
    fz[:, 5 : 5 + H, 3 : 3 + W] = flow_b
    s = np.zeros((128, H + 4, 128), NPBF16)
    for cin in range(2):
        for dh in range(7):
            for dw in range(7):
                # output row r (= buffer row r+2) col c reads fz row r+dh+2, col c+dw
                s[cin * 49 + dh * 7 + dw] = fz[cin, dh : dh + H + 4, dw : dw + 128]
    return s


_MODULE = None


def _get_module():
    global _MODULE
    if _MODULE is None:
        _MODULE = build_module()
    return _MODULE


def make_in_maps(**inputs):
    a = {
        k: np.ascontiguousarray(np.asarray(v), dtype=np.float32)
        for k, v in inputs.items()
    }
    packed = pack_params(
        a["wc1"], a["bc1"], a["wc2"], a["bc2"], a["wf1"], a["bf1"],
        a["wf2"], a["bf2"], a["wo"], a["bo"],
    )
    corr_bf = a["corr"].astype(NPBF16)
    in_maps = []
    for b in range(8):
        m = dict(packed)
        cp = np.zeros((384, H, W), NPBF16)
        cp[0:CIN_CORR] = corr_bf[b]
        m["corr"] = cp
        m["stackh"] = build_stackh(a["flow"][b])
        in_maps.append(m)
    return in_maps, a["flow"]


def assemble_output(results, flow):
    out = np.empty((8, 128, H, W), np.float32)
    for b in range(8):
        out[b, :126] = results[b]["out"]
        out[b, 126:] = flow[b]
    return out


def run(trace=False, **inputs):
    in_maps, flow = make_in_maps(**inputs)
    nc = _get_module()
    res = run_bass_kernel_spmd(nc, in_maps, core_ids=list(range(8)), trace=trace)
    return assemble_output(res.results, flow), res


def kernel(**inputs):
    out, _ = run(trace=False, **inputs)
    return out
